# revision 44
# baseline (speedup 1.0000x reference)
"""CrystalEncoder Trainium2 kernel (v4): all 8 crystals on ONE NeuronCore,
runtime-specialized to the ragged atom counts.

Why one core: in this axon environment each per-device NEFF dispatch carries
~1.2ms of launch overhead and the 8-device dispatch serializes them (~10ms
total — which is what the 9.25ms baseline number actually was). One dispatch
running all 8 crystals sequentially costs 1 launch + the compute.

Ragged specialization: lengths len_c (valid atoms) are in [N/2, N]. The
kernel is BUILT for the lengths observed in the inputs (cached per length
tuple; the build is pure emission, a few hundred ms):
  - j is trimmed to jp_c = len_c rounded up to even (host packs rf rows
    with jp pairs per i-row, so every on-device free dim scales by jp/N);
  - group-1 gate blocks are emitted only for valid i-rows (8-row
    granularity on the last block), and the node update only touches
    the first lp_c columns.
Invalid j inside jp contribute zero via h_j = 0 (padding embedding row);
invalid i inside lp are masked by maskF after the node update.

Per crystal (N=256, H=128, BINS=40, NL=2):
  1. rf64 slice: 16 fills x 4 rows (d^2/d x 2 i-groups), fill = 8 i-rows
     x jp pairs per group (f32r, host-computed).
  2. RBF exponents via K=64 matmuls (cE64 = 16 per-fill selector blocks),
     Exp bias -g*c_k^2 -> rbfT [128, 128*jp] bf16 (groups at partitions
     0/64, same free column = same (i_local, j) pair of each group).
  3. Per layer: gate matmuls (K=40 bf16, <=512-free, psum 8-i-row chunks);
     softplus = Exp then Ln(1+x) (one natural_log_exp table set); DVE 2x
     bf16 multiply by broadcast h_j + add-halves + reduce -> aggT; node
     update zT = node_w^T @ aggT + Silu + residual + mask.
  4. sum over atoms -> sumh column; one [H, 8] output DMA at the end.

Software pipelining: crystal c's layer-2 node update is deferred until
after crystal c+1's RBF stage, and layer-2's first two gate blocks are
produced before layer-1's node update, so ACT (the bottleneck engine)
never waits on DVE reduce tails. All element-wise work is on DVE (GpSimd
tensor ops are Q7 software at ~0.42 efficiency on real HW).

Sync discipline: this walrus build supports at most ONE semaphore wait per
instruction; _install_wait_splitter() splits multi-wait instructions with
same-engine NoOp carriers.
"""

import numpy as np
import ml_dtypes

B, N, H, LAT, NL, BINS = 8, 256, 128, 64, 2, 40
VMAX = 8.0
GAMMA = 1.0 / (VMAX / BINS) ** 2  # 25.0

G = 2                  # i-groups; bins at partition offsets 0 / 64
IPG = N // G           # 128 i-rows per group
NFILL = 16             # rf fills per crystal (8 i-rows per group each)
IPF = 8                # i-rows per fill per group
IPB = 32               # i-rows per gate block
IPC = 8                # i-rows per PSUM chunk
MMF = 512              # matmul free size (hard ISA limit)

_CACHE = {}


def _install_wait_splitter():
    """This walrus build supports at most ONE semaphore wait per ISA
    instruction. Split every multi-wait instruction by inserting same-engine
    NoOp carriers, each holding one of the waits, immediately before it."""
    import bass_rust
    import concourse.tile as tile
    from concourse import mybir

    if getattr(tile.TileContext, "_wait_split_installed", False):
        return
    orig = tile.TileContext._lower_ordered_insts
    counter = [0]

    def patched(self, ordered):
        for insts in ordered.values():
            newl = []
            for inst in insts:
                si = inst.sync_info
                ow = list(si.on_wait) if (si is not None and si.on_wait) else []
                if len(ow) > 1 and inst.engine != mybir.EngineType.Unassigned:
                    for w in ow[:-1]:
                        counter[0] += 1
                        nop = bass_rust.InstNoOp(
                            name=f"wsplit_{counter[0]}", ins=[], outs=[]
                        )
                        nop.engine = inst.engine
                        nop.sync_info = bass_rust.SyncInfo(
                            on_wait=[w], on_update=[]
                        )
                        newl.append(nop)
                    inst.sync_info = bass_rust.SyncInfo(
                        on_wait=[ow[-1]], on_update=list(si.on_update or [])
                    )
                newl.append(inst)
            insts[:] = newl
        return orig(self, ordered)

    tile.TileContext._lower_ordered_insts = patched

    def patched_dab(self, tick_clock, wait_clock):
        from concourse.vector_clock import ScopedClock

        probe = self.nc.sync.nop()
        wait_clock.add_sem_waits(
            probe.ins, ScopedClock({None: tick_clock.global_clock})
        )
        si = probe.ins.sync_info
        ow = list(si.on_wait) if (si is not None and si.on_wait) else []
        if len(ow) > 1:
            probe.ins.sync_info = bass_rust.SyncInfo(
                on_wait=[ow[0]], on_update=list(si.on_update or [])
            )
            for w in ow[1:]:
                n2 = self.nc.sync.nop()
                n2.ins.sync_info = bass_rust.SyncInfo(on_wait=[w], on_update=[])
        self.nc.sync.drain()
        self.nc.all_engine_barrier()
        popped = self.nc._tile_sem_poison_stack.pop()
        assert popped is self._sem_poison
        self.nc.clear_and_free_semaphores(list(self.sems.allocated().values()))
        self.nc.all_engine_barrier()

    tile.TileContext._drain_and_barrier = patched_dab
    tile.TileContext._wait_split_installed = True


def _crystal_geom(length):
    """Per-crystal specialization: (jp, blocks, lp).

    jp: j columns kept (even). blocks: [(g, i0_local, rows)] gate blocks —
    group 0 always 4x32 rows, group 1 in 32-row blocks plus an 8-granular
    remainder. lp = 128 + padded group-1 rows (i columns computed)."""
    length = int(length)
    jp = min(N, length + (length & 1))
    g1 = max(0, min(IPG, length - IPG))
    g1p = -(-g1 // IPC) * IPC
    blocks = [(0, i0, IPB) for i0 in range(0, IPG, IPB)]
    full, rem = divmod(g1p, IPB)
    for k in range(full):
        blocks.append((1, k * IPB, IPB))
    if rem:
        blocks.append((1, full * IPB, rem))
    lp = IPG + g1p
    return jp, blocks, lp


def _build_nc(lengths, poly=False):
    import concourse.bass as bass
    import concourse.tile as tile
    from concourse import mybir

    _install_wait_splitter()

    F32 = mybir.dt.float32
    F32R = mybir.dt.float32r
    BF16 = mybir.dt.bfloat16
    AF = mybir.ActivationFunctionType
    X = mybir.AxisListType
    ALU = mybir.AluOpType
    POOL = mybir.EngineType.Pool
    SP = mybir.EngineType.SP

    nc = bass.Bass("TRN2", target_bir_lowering=False, debug=False)

    def dep_nop(engine, aps):
        """Engine-local nop reading `aps`: pulls their producers' ticks into
        the engine's observed clock so later real instructions need at most
        one new semaphore wait."""
        nop = engine.nop(hint="dep").ins
        nop.ins = [engine.lower_ap(ap) for ap in aps]
        return nop

    NCR = len(lengths)
    rf_offs = [0]
    for c in range(NCR):
        jp_c, _, _ = _crystal_geom(lengths[c])
        rf_offs.append(rf_offs[-1] + IPF * jp_c)
    d_rf = nc.dram_tensor("rf64", [64, rf_offs[-1]], F32R,
                          kind="ExternalInput")
    d_cE = nc.dram_tensor("cE64", [64, NFILL * H], F32R, kind="ExternalInput")
    d_cbias = nc.dram_tensor("cbias", [H, 1], F32, kind="ExternalInput")
    d_ewR = nc.dram_tensor("ewR", [H, NL * H], BF16, kind="ExternalInput")
    d_ebT = nc.dram_tensor("ebT", [H, NL], F32, kind="ExternalInput")
    d_nwT = nc.dram_tensor("nwT", [H, NL * H], F32, kind="ExternalInput")
    d_nbT = nc.dram_tensor("nbT", [H, NL], F32, kind="ExternalInput")
    d_nbTn = nc.dram_tensor("nbTn", [H, NL], F32, kind="ExternalInput")
    d_c2b = nc.dram_tensor("c2bT", [H, NL], F32, kind="ExternalInput")
    d_h0T = nc.dram_tensor("h0T", [H, NCR * N], F32, kind="ExternalInput")
    d_maskR = nc.dram_tensor("maskR", [1, NCR * N], BF16, kind="ExternalInput")
    d_sumh = nc.dram_tensor("sumh", [H, NCR], F32, kind="ExternalOutput")

    with tile.TileContext(nc) as tc:
        with tc.tile_pool(name="consts", bufs=1) as consts, \
             tc.tile_pool(name="rfp", bufs=1) as rfp, \
             tc.tile_pool(name="lay", bufs=2) as lay, \
             tc.tile_pool(name="gxp", bufs=2) as gxp, \
             tc.tile_pool(name="gtp", bufs=2) as gtp, \
             tc.tile_pool(name="ppp", bufs=1) as ppp, \
             tc.tile_pool(name="tmp", bufs=1) as tmpp, \
             tc.tile_pool(name="ps", bufs=2, space="PSUM") as ps:
            kwp = dict(forced_dma_engine=POOL)
            kws = dict(forced_dma_engine=SP)
            # Pool queue stays short so crystal 0's rf DMA lands early;
            # everything bulky or late-needed goes via the SP queue.
            t_cE = consts.tile_from(d_cE[:], **kws)
            t_cbias = consts.tile_from(d_cbias[:], **kwp)
            t_ebT = consts.tile_from(d_ebT[:], **kwp)
            t_nbT = consts.tile_from(d_nbT[:], **kwp)
            t_nbTn = consts.tile_from(d_nbTn[:], **kwp)
            t_c2b = consts.tile_from(d_c2b[:], **kwp)
            t_ewR = consts.tile_from(d_ewR[:], **kws)
            t_nwT = consts.tile_from(d_nwT[:], **kws)
            t_h = consts.tile_from(d_h0T[:], **kws)
            t_maskR = consts.tile_from(d_maskR[:], **kws)
            t_ones = consts.tile([1, H], BF16, tag="ones")
            t_maskF = consts.tile([H, NCR * N], BF16, tag="maskF")

            rbfT = consts.tile([H, IPG * N], BF16)
            sumh = consts.tile([H, NCR], F32, tag="sumh")

            dep_nop(nc.tensor, [t_cE[:], t_ewR[:], t_nwT[:], t_maskR[:]])
            dep_nop(nc.scalar, [t_cbias[:], t_ebT[:], t_nbT[:], t_nbTn[:]])
            dep_nop(nc.vector, [t_c2b[:]])
            dep_nop(nc.vector, [t_h[:]])

            nc.vector.memset(t_ones[:], 1.0)
            dep_nop(nc.tensor, [t_ones[:]])

            def expand_mask():
                """Expand the mask row to all H partitions: ones^T @ maskR
                via K=1 matmuls, copied out of PSUM on DVE. Emitted after
                the first crystal's RBF stage so it never delays the first
                exponent matmuls (maskR is the last const DMA to land);
                it is only needed at the first node update."""
                for q in range(NCR * N // (IPC * N)):
                    mp = ps.tile([H, IPC * N], F32, tag="ps")
                    for s in range(IPC * N // MMF):
                        f0 = q * IPC * N + s * MMF
                        nc.tensor.matmul(
                            mp[:, s * MMF:(s + 1) * MMF], t_ones[:],
                            t_maskR[:, f0:f0 + MMF], start=True, stop=True,
                        )
                    nc.vector.tensor_copy(
                        t_maskF[:, q * IPC * N:(q + 1) * IPC * N], mp[:])

            def stage2(c, jp):
                """RBF table build for crystal c: rf DMA, K=64 exponent
                matmuls per fill, Exp -> rbfT[:, :128*jp]."""
                fw = IPF * jp                       # free width per fill
                rf = rfp.tile([64, IPF * N], F32R, tag="rf")
                nc.gpsimd.dma_start(
                    out=rf[:, :fw],
                    in_=d_rf[:, rf_offs[c]:rf_offs[c] + fw])
                dep_nop(nc.tensor, [rf[:]])
                for f in range(NFILL):
                    e = ps.tile([H, IPC * N], F32, tag="ps")
                    for s in range(-(-fw // MMF)):
                        w = min(MMF, fw - s * MMF)
                        nc.tensor.matmul(
                            e[:, s * MMF:s * MMF + w],
                            t_cE[:, f * H:(f + 1) * H],
                            rf[:, s * MMF:s * MMF + w],
                            start=True, stop=True,
                        )
                    nc.scalar.activation(
                        rbfT[:, f * fw:(f + 1) * fw], e[:, :fw], AF.Exp,
                        bias=t_cbias[:],
                    )

            def gate_produce(l, blk, jp):
                """Gate matmuls + Exp + Ln for one (g, i0_local, rows)
                block."""
                g, i0l, rows = blk
                bw = rows * jp                      # block free width
                lf = i0l * jp
                rp = min(rows, (IPC * N) // jp)     # i-rows per PSUM chunk
                gx = gxp.tile([H, IPB * N], BF16, tag="gx")
                done = 0
                while done < rows:
                    r = min(rp, rows - done)
                    cw = r * jp
                    gp = ps.tile([H, IPC * N], F32, tag="ps")
                    for s in range(-(-cw // MMF)):
                        w = min(MMF, cw - s * MMF)
                        f0 = lf + done * jp + s * MMF
                        nc.tensor.matmul(
                            gp[:, s * MMF:s * MMF + w],
                            t_ewR[64 * g:64 * g + BINS, l * H:(l + 1) * H],
                            rbfT[64 * g:64 * g + BINS, f0:f0 + w],
                            start=True, stop=True,
                        )
                    nc.scalar.activation(
                        gx[:, done * jp:done * jp + cw], gp[:, :cw], AF.Exp,
                        bias=t_ebT[:, l:l + 1],
                    )
                    done += r
                gt = gtp.tile([H, IPB * N], BF16, tag="gt")
                nc.scalar.activation(gt[:, :bw], gx[:, :bw], AF.Ln, bias=1.0)
                return gt

            def gate_consume(gt, blk, jp, hmr, aggT, split=False):
                """DVE: pp = gt * h_j; add j-halves; reduce -> aggT cols."""
                g, i0l, rows = blk
                i0 = g * IPG + i0l
                subs = (rows // IPC) if split else 1
                rw = rows // subs
                w = rw * jp
                pp = ppp.tile([H, IPB * N], BF16, tag="pp")
                tm = tmpp.tile([H, IPB * N // 2], BF16, tag="tm")
                for s in range(subs):
                    sl_ = slice(s * w, (s + 1) * w)
                    ppv = pp[:, sl_].rearrange("p (r c) -> p r c", c=jp)
                    nc.vector.tensor_mul(
                        ppv,
                        gt[:, sl_].rearrange("p (r c) -> p r c", c=jp),
                        hmr[:, None, :jp].broadcast_to([H, rw, jp]),
                    )
                    tmv = tm[:, s * w // 2:(s + 1) * w // 2].rearrange(
                        "p (r c) -> p r c", c=jp // 2)
                    nc.vector.tensor_add(
                        tmv, ppv[:, :, 0:jp // 2], ppv[:, :, jp // 2:jp])
                    nc.vector.reduce_sum(
                        out=aggT[:, i0 + s * rw:i0 + (s + 1) * rw],
                        in_=tmv, axis=X.X,
                    )

            def gate_poly(l, blk, jp, hmr, aggT):
                """Gate block on PE+DVE only (no ACT): softplus(x) ~=
                ln2 + x/2 + x^2/8, valid because the host checked that
                max|x| over the d-grid envelope is small. Offloads work
                from the saturated Scalar engine to DVE headroom."""
                g, i0l, rows = blk
                bw = rows * jp
                lf = i0l * jp
                rp = min(rows, (IPC * N) // jp)
                CN = IPC * N
                sc = gxp.tile([H, IPB * N], BF16, tag="gx")
                pp = ppp.tile([H, IPB * N], BF16, tag="pp")
                done = 0
                while done < rows:
                    r = min(rp, rows - done)
                    cw = r * jp
                    gp = ps.tile([H, CN], F32, tag="ps")
                    for s in range(-(-cw // MMF)):
                        w = min(MMF, cw - s * MMF)
                        f0 = lf + done * jp + s * MMF
                        nc.tensor.matmul(
                            gp[:, s * MMF:s * MMF + w],
                            t_ewR[64 * g:64 * g + BINS, l * H:(l + 1) * H],
                            rbfT[64 * g:64 * g + BINS, f0:f0 + w],
                            start=True, stop=True,
                        )
                    a = sc[:, 0:cw]
                    sq = sc[:, CN:CN + cw]
                    t = sc[:, 2 * CN:2 * CN + cw]
                    # a = (y + b)/sqrt(8); sq = a^2 = x^2/8
                    nc.vector.tensor_scalar(
                        a, gp[:, :cw], t_ebT[:, l:l + 1], 0.35355339,
                        op0=ALU.add, op1=ALU.mult)
                    nc.vector.tensor_mul(sq, a, a)
                    # t = y/2 + x^2/8
                    nc.vector.scalar_tensor_tensor(
                        t, gp[:, :cw], 0.5, sq, op0=ALU.mult, op1=ALU.add)
                    # pp = (t + (ln2 + b/2)) * h_j
                    ppv = pp[:, done * jp:done * jp + cw].rearrange(
                        "p (r c) -> p r c", c=jp)
                    nc.vector.scalar_tensor_tensor(
                        ppv, t.rearrange("p (r c) -> p r c", c=jp),
                        t_c2b[:, l:l + 1],
                        hmr[:, None, :jp].broadcast_to([H, r, jp]),
                        op0=ALU.add, op1=ALU.mult)
                    done += r
                tm = tmpp.tile([H, IPB * N // 2], BF16, tag="tm")
                ppv = pp[:, :bw].rearrange("p (r c) -> p r c", c=jp)
                tmv = tm[:, :bw // 2].rearrange(
                    "p (r c) -> p r c", c=jp // 2)
                nc.vector.tensor_add(
                    tmv, ppv[:, :, 0:jp // 2], ppv[:, :, jp // 2:jp])
                i0 = g * IPG + i0l
                nc.vector.reduce_sum(
                    out=aggT[:, i0:i0 + rows], in_=tmv, axis=X.X)

            def node_update(c, l, aggT, lp):
                """zT = node_w^T @ aggT; h += silu(zT + b); h *= mask.
                silu(z) = z * exp(-ln(1 + exp(-z))) uses only the
                natural_log_exp table set — no ACT table switches.
                Only the first lp columns are computed columns."""
                hsl = slice(c * N, c * N + lp)
                dep_nop(nc.tensor, [aggT[:]])
                zp = ps.tile([H, IPC * N], F32, tag="ps")
                nc.tensor.matmul(
                    zp[:, :lp], t_nwT[:, l * H:(l + 1) * H], aggT[:, :lp],
                    start=True, stop=True,
                )
                # z = zp + node_b (fold bias into the first Exp's scale
                # trick is not possible: bias applies pre-function), so
                # add it on DVE first.
                # clamp first (only dep: zp) so the ACT chain starts after
                # ONE DVE op; the unclamped z for the final multiply is
                # computed in parallel with the ACT chain. silu(z<-30) ~ 0.
                ztc = lay.tile([H, N], F32, tag="ztc")
                nc.vector.tensor_scalar_max(ztc[:, :lp], zp[:, :lp], -30.0)
                zt = lay.tile([H, N], F32, tag="zt")
                nc.vector.tensor_scalar_add(
                    zt[:, :lp], zp[:, :lp], t_nbT[:, l:l + 1])
                u = lay.tile([H, N], F32, tag="sgu")
                nc.scalar.activation(u[:, :lp], ztc[:, :lp], AF.Exp,
                                     scale=-1.0, bias=t_nbTn[:, l:l + 1])
                w = lay.tile([H, N], F32, tag="sgw")
                nc.scalar.activation(w[:, :lp], u[:, :lp], AF.Ln, bias=1.0)
                sg = lay.tile([H, N], F32, tag="sgs")
                nc.scalar.activation(sg[:, :lp], w[:, :lp], AF.Exp,
                                     scale=-1.0)
                sl = lay.tile([H, N], F32, tag="sil")
                nc.vector.tensor_mul(sl[:, :lp], zt[:, :lp], sg[:, :lp])
                h2 = lay.tile([H, N], F32, tag="h2")
                nc.vector.tensor_add(h2[:, :lp], t_h[:, hsl], sl[:, :lp])
                nc.vector.tensor_mul(t_h[:, hsl], h2[:, :lp],
                                     t_maskF[:, hsl])

            deferred = None  # (c, aggT2, lp) awaiting layer-2 node update

            def finish_crystal(dfr):
                c, aggT2, lp = dfr
                node_update(c, 1, aggT2, lp)
                nc.vector.reduce_sum(
                    out=sumh[:, c:c + 1], in_=t_h[:, c * N:(c + 1) * N],
                    axis=X.X,
                )

            # longest crystals first: the final (un-hideable) reduce tail
            # then belongs to the shortest crystal
            order = sorted(range(NCR), key=lambda c: -int(lengths[c]))
            first = True
            for c in order:
                jp, blocks, lp = _crystal_geom(lengths[c])
                nblk = len(blocks)
                stage2(c, jp)
                if first:
                    expand_mask()
                    first = False
                if deferred is not None:
                    finish_crystal(deferred)
                    deferred = None
                hsl = slice(c * N, (c + 1) * N)
                hmr1 = lay.tile([H, N], BF16, tag="hmr1")
                nc.vector.tensor_copy(hmr1[:], t_h[:, hsl])
                # layer 1
                aggT1 = lay.tile([H, N], F32, tag="agg1")
                for b in range(nblk):
                    gt = gate_produce(0, blocks[b], jp)
                    gate_consume(gt, blocks[b], jp, hmr1, aggT1,
                                 split=(b == nblk - 1))
                # layer 2: produce first two blocks before layer-1 node
                # update so ACT stays busy over the layer-1 reduce tail
                aggT2 = lay.tile([H, N], F32, tag="agg2")
                gt20 = gate_produce(1, blocks[0], jp)
                gt21 = gate_produce(1, blocks[1], jp)
                node_update(c, 0, aggT1, lp)
                hmr2 = lay.tile([H, N], BF16, tag="hmr2")
                nc.vector.tensor_copy(hmr2[:], t_h[:, hsl])
                gate_consume(gt20, blocks[0], jp, hmr2, aggT2)
                gate_consume(gt21, blocks[1], jp, hmr2, aggT2)
                for b in range(2, nblk):
                    if poly and jp >= 160 and nblk >= 5 and b in (2, 3):
                        gate_poly(1, blocks[b], jp, hmr2, aggT2)
                    else:
                        gt = gate_produce(1, blocks[b], jp)
                        gate_consume(gt, blocks[b], jp, hmr2, aggT2,
                                     split=(b == nblk - 1))
                deferred = (c, aggT2, lp)

            finish_crystal(deferred)
            nc.gpsimd.dma_start(out=d_sumh[:], in_=sumh[:])

    return nc


def _get_nc(lengths, poly=False):
    key = (tuple(int(x) for x in lengths), bool(poly))
    if key not in _CACHE:
        _CACHE[key] = _build_nc(key[0], poly=key[1])
    return _CACHE[key]


def _shared_inputs(edge_w, edge_b, node_w, node_b):
    centers = np.linspace(0.0, VMAX, BINS).astype(np.float64)
    # cE64: 16 per-fill selector blocks. Fill f uses rf rows 4f..4f+3:
    # row 4f+2g+0 = d^2 of group g, row 4f+2g+1 = d of group g.
    cE = np.zeros((64, NFILL * H), np.float32)
    for f in range(NFILL):
        for g in range(G):
            col0 = f * H + 64 * g
            cE[4 * f + 2 * g + 0, col0:col0 + BINS] = -GAMMA
            cE[4 * f + 2 * g + 1, col0:col0 + BINS] = 2.0 * GAMMA * centers
    cbias = np.zeros((H, 1), np.float32)
    ewR = np.zeros((H, NL * H), np.float32)
    for g in range(G):
        cbias[64 * g:64 * g + BINS, 0] = -GAMMA * centers * centers
        for l in range(NL):
            ewR[64 * g:64 * g + BINS, l * H:(l + 1) * H] = edge_w[l]
    ewR = ewR.astype(ml_dtypes.bfloat16)
    ebT = np.ascontiguousarray(edge_b.T).astype(np.float32)      # [H, NL]
    nwT = np.concatenate([node_w[l] for l in range(NL)], axis=1)
    nwT = np.ascontiguousarray(nwT).astype(np.float32)           # [H, NL*H]
    nbT = np.ascontiguousarray(node_b.T).astype(np.float32)      # [H, NL]
    return dict(cE64=cE, cbias=cbias, ewR=ewR, ebT=ebT, nwT=nwT, nbT=nbT,
                nbTn=-nbT, c2bT=np.float32(np.log(2.0)) + 0.5 * ebT)


def make_in_maps(atom_types, frac_coords, lattice, mask, emb_table,
                 edge_w, edge_b, node_w, node_b):
    shared = _shared_inputs(edge_w, edge_b, node_w, node_b)
    lengths = mask.sum(1).astype(int)
    cart = np.einsum('bnd,bde->bne', frac_coords, lattice).astype(np.float32)
    nsq = (cart * cart).sum(-1)                                   # (B, N)
    d2 = (nsq[:, :, None] + nsq[:, None, :]
          - 2.0 * np.einsum('bid,bjd->bij', cart, cart))
    d2 = np.maximum(d2, 0.0).astype(np.float32) + np.float32(1e-6)
    d = np.sqrt(d2)
    # rf64 [64, sum(8*jp_c)]: crystal c at its packed offset; fill f rows
    # 4f+2g+{0,1} = (d^2, d) of group g, i-rows [8f, 8f+8), j < jp_c,
    # row-major over (i, j).
    offs = [0]
    jps = []
    for c in range(B):
        jp, _, _ = _crystal_geom(lengths[c])
        jps.append(jp)
        offs.append(offs[-1] + IPF * jp)
    rf = np.zeros((64, offs[-1]), np.float32)
    for c in range(B):
        jp = jps[c]
        fw = IPF * jp
        csl = slice(offs[c], offs[c] + fw)
        for f in range(NFILL):
            for g in range(G):
                i0 = g * IPG + f * IPF
                rf[4 * f + 2 * g + 0, csl] = \
                    d2[c, i0:i0 + IPF, :jp].reshape(-1)
                rf[4 * f + 2 * g + 1, csl] = \
                    d[c, i0:i0 + IPF, :jp].reshape(-1)
    types = np.where(mask, atom_types, 0).astype(np.int64)        # (B, N)
    h0 = emb_table[types]                                         # (B, N, H)
    h0T = np.ascontiguousarray(
        h0.transpose(2, 0, 1).reshape(H, B * N)).astype(np.float32)
    maskR = mask.astype(np.float32).reshape(1, B * N).astype(
        ml_dtypes.bfloat16)
    return [dict(rf64=rf, h0T=h0T, maskR=maskR, **shared)]


def _ensure_ntff_hook():
    """run_bass_kernel_spmd(trace=True) imports antenv.axon_hooks, which
    some agent images lack. If it's missing, register the equivalent hook
    from the boot module so a BASS_TRACE=1 run profiles instead of
    crashing. No-op when the real module exists."""
    import sys as _sys
    try:
        import antenv.axon_hooks  # noqa: F401
        return
    except ImportError:
        pass
    try:
        import types as _types
        import antenv  # noqa: F401
        import trn_agent_boot.trn_boot as _tb
        hook = _tb._ntff_profile_via_ctypes('/opt/axon/libaxon_pjrt.so')
        mod = _types.ModuleType('antenv.axon_hooks')
        mod.get_axon_ntff_profile_hook = lambda: hook
        mod.set_axon_ntff_profile_hook = lambda h: None
        _sys.modules['antenv.axon_hooks'] = mod
    except Exception:
        pass


def kernel(**inputs):
    from concourse.bass_utils import run_bass_kernel_spmd

    _ensure_ntff_hook()

    atom_types = np.asarray(inputs["atom_types"])
    frac_coords = np.asarray(inputs["frac_coords"], np.float32)
    lattice = np.asarray(inputs["lattice"], np.float32)
    mask = np.asarray(inputs["mask"]).astype(bool)
    emb_table = np.asarray(inputs["emb_table"], np.float32)
    edge_w = np.asarray(inputs["edge_w"], np.float32)
    edge_b = np.asarray(inputs["edge_b"], np.float32)
    node_w = np.asarray(inputs["node_w"], np.float32)
    node_b = np.asarray(inputs["node_b"], np.float32)
    mu_w = np.asarray(inputs["mu_w"], np.float32)
    mu_b = np.asarray(inputs["mu_b"], np.float32)
    var_w = np.asarray(inputs["var_w"], np.float32)
    var_b = np.asarray(inputs["var_b"], np.float32)

    lengths = mask.sum(1).astype(int)
    # polynomial-softplus validity: bound max|gate_pre| by the max over a
    # d-grid of the |edge_w|-weighted RBF envelope (plus |edge_b|)
    dg = np.linspace(0.0, 20.0, 512)
    centers = np.linspace(0.0, VMAX, BINS)
    env = np.exp(-GAMMA * (dg[:, None] - centers[None, :]) ** 2)
    bound = max(
        float((env @ np.abs(edge_w[l])).max()) for l in range(NL)
    ) + float(np.abs(edge_b).max())
    poly = bound < 1.0
    nc = _get_nc(lengths, poly=poly)
    _CACHE["last_nc"] = nc
    in_maps = make_in_maps(atom_types, frac_coords, lattice, mask, emb_table,
                           edge_w, edge_b, node_w, node_b)
    res = run_bass_kernel_spmd(nc, in_maps, core_ids=[0])
    sum_h = np.ascontiguousarray(res.results[0]["sumh"].T)        # (B, H)
    n_valid = mask.sum(1).astype(np.float32)
    g = sum_h / (n_valid[:, None] + 1e-6)
    mu = (g @ mu_w + mu_b).astype(np.float32)
    log_var = (g @ var_w + var_b).astype(np.float32)
    return mu, log_var


# revision 45
# speedup vs baseline: 1.1693x; 1.1693x over previous
"""CrystalEncoder Trainium2 kernel (v4): all 8 crystals on ONE NeuronCore,
runtime-specialized to the ragged atom counts.

Why one core: in this axon environment each per-device NEFF dispatch carries
~1.2ms of launch overhead and the 8-device dispatch serializes them (~10ms
total — which is what the 9.25ms baseline number actually was). One dispatch
running all 8 crystals sequentially costs 1 launch + the compute.

Ragged specialization: lengths len_c (valid atoms) are in [N/2, N]. The
kernel is BUILT for the lengths observed in the inputs (cached per length
tuple; the build is pure emission, a few hundred ms):
  - j is trimmed to jp_c = len_c rounded up to even (host packs rf rows
    with jp pairs per i-row, so every on-device free dim scales by jp/N);
  - group-1 gate blocks are emitted only for valid i-rows (8-row
    granularity on the last block), and the node update only touches
    the first lp_c columns.
Invalid j inside jp contribute zero via h_j = 0 (padding embedding row);
invalid i inside lp are masked by maskF after the node update.

Per crystal (N=256, H=128, BINS=40, NL=2):
  1. rf64 slice: 16 fills x 4 rows (d^2/d x 2 i-groups), fill = 8 i-rows
     x jp pairs per group (f32r, host-computed).
  2. RBF exponents via K=64 matmuls (cE64 = 16 per-fill selector blocks),
     Exp bias -g*c_k^2 -> rbfT [128, 128*jp] bf16 (groups at partitions
     0/64, same free column = same (i_local, j) pair of each group).
  3. Per layer: gate matmuls (K=40 bf16, <=512-free, psum 8-i-row chunks);
     softplus = Exp then Ln(1+x) (one natural_log_exp table set); DVE 2x
     bf16 multiply by broadcast h_j + add-halves + reduce -> aggT; node
     update zT = node_w^T @ aggT + Silu + residual + mask.
  4. sum over atoms -> sumh column; one [H, 8] output DMA at the end.

Software pipelining: crystal c's layer-2 node update is deferred until
after crystal c+1's RBF stage, and layer-2's first two gate blocks are
produced before layer-1's node update, so ACT (the bottleneck engine)
never waits on DVE reduce tails. All element-wise work is on DVE (GpSimd
tensor ops are Q7 software at ~0.42 efficiency on real HW).

Sync discipline: this walrus build supports at most ONE semaphore wait per
instruction; _install_wait_splitter() splits multi-wait instructions with
same-engine NoOp carriers.
"""

import numpy as np
import ml_dtypes

B, N, H, LAT, NL, BINS = 8, 256, 128, 64, 2, 40
VMAX = 8.0
GAMMA = 1.0 / (VMAX / BINS) ** 2  # 25.0

G = 2                  # i-groups; bins at partition offsets 0 / 64
IPG = N // G           # 128 i-rows per group
NFILL = 16             # rf fills per crystal (8 i-rows per group each)
IPF = 8                # i-rows per fill per group
IPB = 32               # i-rows per gate block
IPC = 8                # i-rows per PSUM chunk
MMF = 512              # matmul free size (hard ISA limit)

_CACHE = {}


def _install_wait_splitter():
    """This walrus build supports at most ONE semaphore wait per ISA
    instruction. Split every multi-wait instruction by inserting same-engine
    NoOp carriers, each holding one of the waits, immediately before it."""
    import bass_rust
    import concourse.tile as tile
    from concourse import mybir

    if getattr(tile.TileContext, "_wait_split_installed", False):
        return
    orig = tile.TileContext._lower_ordered_insts
    counter = [0]

    def patched(self, ordered):
        for insts in ordered.values():
            newl = []
            for inst in insts:
                si = inst.sync_info
                ow = list(si.on_wait) if (si is not None and si.on_wait) else []
                if len(ow) > 1 and inst.engine != mybir.EngineType.Unassigned:
                    for w in ow[:-1]:
                        counter[0] += 1
                        nop = bass_rust.InstNoOp(
                            name=f"wsplit_{counter[0]}", ins=[], outs=[]
                        )
                        nop.engine = inst.engine
                        nop.sync_info = bass_rust.SyncInfo(
                            on_wait=[w], on_update=[]
                        )
                        newl.append(nop)
                    inst.sync_info = bass_rust.SyncInfo(
                        on_wait=[ow[-1]], on_update=list(si.on_update or [])
                    )
                newl.append(inst)
            insts[:] = newl
        return orig(self, ordered)

    tile.TileContext._lower_ordered_insts = patched

    def patched_dab(self, tick_clock, wait_clock):
        from concourse.vector_clock import ScopedClock

        probe = self.nc.sync.nop()
        wait_clock.add_sem_waits(
            probe.ins, ScopedClock({None: tick_clock.global_clock})
        )
        si = probe.ins.sync_info
        ow = list(si.on_wait) if (si is not None and si.on_wait) else []
        if len(ow) > 1:
            probe.ins.sync_info = bass_rust.SyncInfo(
                on_wait=[ow[0]], on_update=list(si.on_update or [])
            )
            for w in ow[1:]:
                n2 = self.nc.sync.nop()
                n2.ins.sync_info = bass_rust.SyncInfo(on_wait=[w], on_update=[])
        self.nc.sync.drain()
        self.nc.all_engine_barrier()
        popped = self.nc._tile_sem_poison_stack.pop()
        assert popped is self._sem_poison
        self.nc.clear_and_free_semaphores(list(self.sems.allocated().values()))
        self.nc.all_engine_barrier()

    tile.TileContext._drain_and_barrier = patched_dab
    tile.TileContext._wait_split_installed = True


def _crystal_geom(length):
    """Per-crystal specialization: (jp, blocks, lp).

    jp: j columns kept (even). blocks: [(g, i0_local, rows)] gate blocks —
    group 0 always 4x32 rows, group 1 in 32-row blocks plus an 8-granular
    remainder. lp = 128 + padded group-1 rows (i columns computed)."""
    length = int(length)
    jp = min(N, length + (length & 1))
    g1 = max(0, min(IPG, length - IPG))
    g1p = -(-g1 // IPC) * IPC
    blocks = [(0, i0, IPB) for i0 in range(0, IPG, IPB)]
    full, rem = divmod(g1p, IPB)
    for k in range(full):
        blocks.append((1, k * IPB, IPB))
    if rem:
        blocks.append((1, full * IPB, rem))
    lp = IPG + g1p
    return jp, blocks, lp


def _build_nc(lengths):
    import concourse.bass as bass
    import concourse.tile as tile
    from concourse import mybir

    _install_wait_splitter()

    F32 = mybir.dt.float32
    F32R = mybir.dt.float32r
    BF16 = mybir.dt.bfloat16
    AF = mybir.ActivationFunctionType
    X = mybir.AxisListType
    POOL = mybir.EngineType.Pool
    SP = mybir.EngineType.SP

    nc = bass.Bass("TRN2", target_bir_lowering=False, debug=False)

    def dep_nop(engine, aps):
        """Engine-local nop reading `aps`: pulls their producers' ticks into
        the engine's observed clock so later real instructions need at most
        one new semaphore wait."""
        nop = engine.nop(hint="dep").ins
        nop.ins = [engine.lower_ap(ap) for ap in aps]
        return nop

    NCR = len(lengths)
    rf_offs = [0]
    for c in range(NCR):
        jp_c, _, _ = _crystal_geom(lengths[c])
        rf_offs.append(rf_offs[-1] + IPF * jp_c)
    d_rf = nc.dram_tensor("rf64", [64, rf_offs[-1]], F32R,
                          kind="ExternalInput")
    d_cE = nc.dram_tensor("cE64", [64, NFILL * H], F32R, kind="ExternalInput")
    d_cbias = nc.dram_tensor("cbias", [H, 1], F32, kind="ExternalInput")
    d_ewR = nc.dram_tensor("ewR", [H, NL * H], BF16, kind="ExternalInput")
    d_ebT = nc.dram_tensor("ebT", [H, NL], F32, kind="ExternalInput")
    d_nwT = nc.dram_tensor("nwT", [H, NL * H], F32, kind="ExternalInput")
    d_nbT = nc.dram_tensor("nbT", [H, NL], F32, kind="ExternalInput")
    d_nbTn = nc.dram_tensor("nbTn", [H, NL], F32, kind="ExternalInput")
    d_h0T = nc.dram_tensor("h0T", [H, NCR * N], F32, kind="ExternalInput")
    d_maskR = nc.dram_tensor("maskR", [1, NCR * N], BF16, kind="ExternalInput")
    d_sumh = nc.dram_tensor("sumh", [H, NCR], F32, kind="ExternalOutput")

    with tile.TileContext(nc) as tc:
        with tc.tile_pool(name="consts", bufs=1) as consts, \
             tc.tile_pool(name="rfp", bufs=1) as rfp, \
             tc.tile_pool(name="lay", bufs=2) as lay, \
             tc.tile_pool(name="gxp", bufs=2) as gxp, \
             tc.tile_pool(name="gtp", bufs=2) as gtp, \
             tc.tile_pool(name="ppp", bufs=1) as ppp, \
             tc.tile_pool(name="tmp", bufs=1) as tmpp, \
             tc.tile_pool(name="ps", bufs=2, space="PSUM") as ps:
            kwp = dict(forced_dma_engine=POOL)
            kws = dict(forced_dma_engine=SP)
            # Pool queue stays short so crystal 0's rf DMA lands early;
            # everything bulky or late-needed goes via the SP queue.
            t_cE = consts.tile_from(d_cE[:], **kws)
            t_cbias = consts.tile_from(d_cbias[:], **kwp)
            t_ebT = consts.tile_from(d_ebT[:], **kwp)
            t_nbT = consts.tile_from(d_nbT[:], **kwp)
            t_nbTn = consts.tile_from(d_nbTn[:], **kwp)
            t_ewR = consts.tile_from(d_ewR[:], **kws)
            t_nwT = consts.tile_from(d_nwT[:], **kws)
            t_h = consts.tile_from(d_h0T[:], **kws)
            t_maskR = consts.tile_from(d_maskR[:], **kws)
            t_ones = consts.tile([1, H], BF16, tag="ones")
            t_maskF = consts.tile([H, NCR * N], BF16, tag="maskF")

            rbfT = consts.tile([H, IPG * N], BF16)
            sumh = consts.tile([H, NCR], F32, tag="sumh")

            dep_nop(nc.tensor, [t_cE[:], t_ewR[:], t_nwT[:], t_maskR[:]])
            dep_nop(nc.scalar, [t_cbias[:], t_ebT[:], t_nbT[:], t_nbTn[:]])
            dep_nop(nc.vector, [t_h[:]])

            nc.vector.memset(t_ones[:], 1.0)
            dep_nop(nc.tensor, [t_ones[:]])

            def expand_mask():
                """Expand the mask row to all H partitions: ones^T @ maskR
                via K=1 matmuls, copied out of PSUM on DVE. Emitted after
                the first crystal's RBF stage so it never delays the first
                exponent matmuls (maskR is the last const DMA to land);
                it is only needed at the first node update."""
                for q in range(NCR * N // (IPC * N)):
                    mp = ps.tile([H, IPC * N], F32, tag="ps")
                    for s in range(IPC * N // MMF):
                        f0 = q * IPC * N + s * MMF
                        nc.tensor.matmul(
                            mp[:, s * MMF:(s + 1) * MMF], t_ones[:],
                            t_maskR[:, f0:f0 + MMF], start=True, stop=True,
                        )
                    nc.vector.tensor_copy(
                        t_maskF[:, q * IPC * N:(q + 1) * IPC * N], mp[:])

            def stage2(c, jp):
                """RBF table build for crystal c: rf DMA, K=64 exponent
                matmuls per fill, Exp -> rbfT[:, :128*jp]."""
                fw = IPF * jp                       # free width per fill
                rf = rfp.tile([64, IPF * N], F32R, tag="rf")
                nc.gpsimd.dma_start(
                    out=rf[:, :fw],
                    in_=d_rf[:, rf_offs[c]:rf_offs[c] + fw])
                dep_nop(nc.tensor, [rf[:]])
                for f in range(NFILL):
                    e = ps.tile([H, IPC * N], F32, tag="ps")
                    for s in range(-(-fw // MMF)):
                        w = min(MMF, fw - s * MMF)
                        nc.tensor.matmul(
                            e[:, s * MMF:s * MMF + w],
                            t_cE[:, f * H:(f + 1) * H],
                            rf[:, s * MMF:s * MMF + w],
                            start=True, stop=True,
                        )
                    nc.scalar.activation(
                        rbfT[:, f * fw:(f + 1) * fw], e[:, :fw], AF.Exp,
                        bias=t_cbias[:],
                    )

            def gate_produce(l, blk, jp):
                """Gate matmuls + Exp + Ln for one (g, i0_local, rows)
                block."""
                g, i0l, rows = blk
                bw = rows * jp                      # block free width
                lf = i0l * jp
                rp = min(rows, (IPC * N) // jp)     # i-rows per PSUM chunk
                gx = gxp.tile([H, IPB * N], BF16, tag="gx")
                done = 0
                while done < rows:
                    r = min(rp, rows - done)
                    cw = r * jp
                    gp = ps.tile([H, IPC * N], F32, tag="ps")
                    for s in range(-(-cw // MMF)):
                        w = min(MMF, cw - s * MMF)
                        f0 = lf + done * jp + s * MMF
                        nc.tensor.matmul(
                            gp[:, s * MMF:s * MMF + w],
                            t_ewR[64 * g:64 * g + BINS, l * H:(l + 1) * H],
                            rbfT[64 * g:64 * g + BINS, f0:f0 + w],
                            start=True, stop=True,
                        )
                    nc.scalar.activation(
                        gx[:, done * jp:done * jp + cw], gp[:, :cw], AF.Exp,
                        bias=t_ebT[:, l:l + 1],
                    )
                    done += r
                gt = gtp.tile([H, IPB * N], BF16, tag="gt")
                nc.scalar.activation(gt[:, :bw], gx[:, :bw], AF.Ln, bias=1.0)
                return gt

            def gate_consume(gt, blk, jp, hmr, aggT, split=False):
                """DVE: pp = gt * h_j; add j-halves; reduce -> aggT cols."""
                g, i0l, rows = blk
                i0 = g * IPG + i0l
                subs = (rows // IPC) if split else 1
                rw = rows // subs
                w = rw * jp
                pp = ppp.tile([H, IPB * N], BF16, tag="pp")
                tm = tmpp.tile([H, IPB * N // 2], BF16, tag="tm")
                for s in range(subs):
                    sl_ = slice(s * w, (s + 1) * w)
                    ppv = pp[:, sl_].rearrange("p (r c) -> p r c", c=jp)
                    nc.vector.tensor_mul(
                        ppv,
                        gt[:, sl_].rearrange("p (r c) -> p r c", c=jp),
                        hmr[:, None, :jp].broadcast_to([H, rw, jp]),
                    )
                    tmv = tm[:, s * w // 2:(s + 1) * w // 2].rearrange(
                        "p (r c) -> p r c", c=jp // 2)
                    nc.vector.tensor_add(
                        tmv, ppv[:, :, 0:jp // 2], ppv[:, :, jp // 2:jp])
                    nc.vector.reduce_sum(
                        out=aggT[:, i0 + s * rw:i0 + (s + 1) * rw],
                        in_=tmv, axis=X.X,
                    )

            def node_update(c, l, aggT, lp):
                """zT = node_w^T @ aggT; h += silu(zT + b); h *= mask.
                silu(z) = z * exp(-ln(1 + exp(-z))) uses only the
                natural_log_exp table set — no ACT table switches.
                Only the first lp columns are computed columns."""
                hsl = slice(c * N, c * N + lp)
                dep_nop(nc.tensor, [aggT[:]])
                zp = ps.tile([H, IPC * N], F32, tag="ps")
                nc.tensor.matmul(
                    zp[:, :lp], t_nwT[:, l * H:(l + 1) * H], aggT[:, :lp],
                    start=True, stop=True,
                )
                # z = zp + node_b (fold bias into the first Exp's scale
                # trick is not possible: bias applies pre-function), so
                # add it on DVE first.
                # clamp first (only dep: zp) so the ACT chain starts after
                # ONE DVE op; the unclamped z for the final multiply is
                # computed in parallel with the ACT chain. silu(z<-30) ~ 0.
                ztc = lay.tile([H, N], F32, tag="ztc")
                nc.vector.tensor_scalar_max(ztc[:, :lp], zp[:, :lp], -30.0)
                zt = lay.tile([H, N], F32, tag="zt")
                nc.vector.tensor_scalar_add(
                    zt[:, :lp], zp[:, :lp], t_nbT[:, l:l + 1])
                u = lay.tile([H, N], F32, tag="sgu")
                nc.scalar.activation(u[:, :lp], ztc[:, :lp], AF.Exp,
                                     scale=-1.0, bias=t_nbTn[:, l:l + 1])
                w = lay.tile([H, N], F32, tag="sgw")
                nc.scalar.activation(w[:, :lp], u[:, :lp], AF.Ln, bias=1.0)
                sg = lay.tile([H, N], F32, tag="sgs")
                nc.scalar.activation(sg[:, :lp], w[:, :lp], AF.Exp,
                                     scale=-1.0)
                sl = lay.tile([H, N], F32, tag="sil")
                nc.vector.tensor_mul(sl[:, :lp], zt[:, :lp], sg[:, :lp])
                h2 = lay.tile([H, N], F32, tag="h2")
                nc.vector.tensor_add(h2[:, :lp], t_h[:, hsl], sl[:, :lp])
                nc.vector.tensor_mul(t_h[:, hsl], h2[:, :lp],
                                     t_maskF[:, hsl])

            deferred = None  # (c, aggT2, lp) awaiting layer-2 node update

            def finish_crystal(dfr):
                c, aggT2, lp = dfr
                node_update(c, 1, aggT2, lp)
                nc.vector.reduce_sum(
                    out=sumh[:, c:c + 1], in_=t_h[:, c * N:(c + 1) * N],
                    axis=X.X,
                )

            # longest crystals first: the final (un-hideable) reduce tail
            # then belongs to the shortest crystal
            order = sorted(range(NCR), key=lambda c: -int(lengths[c]))
            first = True
            for c in order:
                jp, blocks, lp = _crystal_geom(lengths[c])
                nblk = len(blocks)
                stage2(c, jp)
                if first:
                    expand_mask()
                    first = False
                if deferred is not None:
                    finish_crystal(deferred)
                    deferred = None
                hsl = slice(c * N, (c + 1) * N)
                hmr1 = lay.tile([H, N], BF16, tag="hmr1")
                nc.vector.tensor_copy(hmr1[:], t_h[:, hsl])
                # layer 1
                aggT1 = lay.tile([H, N], F32, tag="agg1")
                for b in range(nblk):
                    gt = gate_produce(0, blocks[b], jp)
                    gate_consume(gt, blocks[b], jp, hmr1, aggT1,
                                 split=(b == nblk - 1))
                # layer 2: produce first two blocks before layer-1 node
                # update so ACT stays busy over the layer-1 reduce tail
                aggT2 = lay.tile([H, N], F32, tag="agg2")
                gt20 = gate_produce(1, blocks[0], jp)
                gt21 = gate_produce(1, blocks[1], jp)
                node_update(c, 0, aggT1, lp)
                hmr2 = lay.tile([H, N], BF16, tag="hmr2")
                nc.vector.tensor_copy(hmr2[:], t_h[:, hsl])
                gate_consume(gt20, blocks[0], jp, hmr2, aggT2)
                gate_consume(gt21, blocks[1], jp, hmr2, aggT2)
                for b in range(2, nblk):
                    gt = gate_produce(1, blocks[b], jp)
                    gate_consume(gt, blocks[b], jp, hmr2, aggT2,
                                 split=(b == nblk - 1))
                deferred = (c, aggT2, lp)

            finish_crystal(deferred)
            nc.gpsimd.dma_start(out=d_sumh[:], in_=sumh[:])

    return nc


def _get_nc(lengths):
    key = tuple(int(x) for x in lengths)
    if key not in _CACHE:
        _CACHE[key] = _build_nc(key)
    return _CACHE[key]


def _shared_inputs(edge_w, edge_b, node_w, node_b):
    centers = np.linspace(0.0, VMAX, BINS).astype(np.float64)
    # cE64: 16 per-fill selector blocks. Fill f uses rf rows 4f..4f+3:
    # row 4f+2g+0 = d^2 of group g, row 4f+2g+1 = d of group g.
    cE = np.zeros((64, NFILL * H), np.float32)
    for f in range(NFILL):
        for g in range(G):
            col0 = f * H + 64 * g
            cE[4 * f + 2 * g + 0, col0:col0 + BINS] = -GAMMA
            cE[4 * f + 2 * g + 1, col0:col0 + BINS] = 2.0 * GAMMA * centers
    cbias = np.zeros((H, 1), np.float32)
    ewR = np.zeros((H, NL * H), np.float32)
    for g in range(G):
        cbias[64 * g:64 * g + BINS, 0] = -GAMMA * centers * centers
        for l in range(NL):
            ewR[64 * g:64 * g + BINS, l * H:(l + 1) * H] = edge_w[l]
    ewR = ewR.astype(ml_dtypes.bfloat16)
    ebT = np.ascontiguousarray(edge_b.T).astype(np.float32)      # [H, NL]
    nwT = np.concatenate([node_w[l] for l in range(NL)], axis=1)
    nwT = np.ascontiguousarray(nwT).astype(np.float32)           # [H, NL*H]
    nbT = np.ascontiguousarray(node_b.T).astype(np.float32)      # [H, NL]
    return dict(cE64=cE, cbias=cbias, ewR=ewR, ebT=ebT, nwT=nwT, nbT=nbT,
                nbTn=-nbT)


def make_in_maps(atom_types, frac_coords, lattice, mask, emb_table,
                 edge_w, edge_b, node_w, node_b):
    shared = _shared_inputs(edge_w, edge_b, node_w, node_b)
    lengths = mask.sum(1).astype(int)
    cart = np.einsum('bnd,bde->bne', frac_coords, lattice).astype(np.float32)
    nsq = (cart * cart).sum(-1)                                   # (B, N)
    d2 = (nsq[:, :, None] + nsq[:, None, :]
          - 2.0 * np.einsum('bid,bjd->bij', cart, cart))
    d2 = np.maximum(d2, 0.0).astype(np.float32) + np.float32(1e-6)
    d = np.sqrt(d2)
    # rf64 [64, sum(8*jp_c)]: crystal c at its packed offset; fill f rows
    # 4f+2g+{0,1} = (d^2, d) of group g, i-rows [8f, 8f+8), j < jp_c,
    # row-major over (i, j).
    offs = [0]
    jps = []
    for c in range(B):
        jp, _, _ = _crystal_geom(lengths[c])
        jps.append(jp)
        offs.append(offs[-1] + IPF * jp)
    rf = np.zeros((64, offs[-1]), np.float32)
    for c in range(B):
        jp = jps[c]
        fw = IPF * jp
        csl = slice(offs[c], offs[c] + fw)
        for f in range(NFILL):
            for g in range(G):
                i0 = g * IPG + f * IPF
                rf[4 * f + 2 * g + 0, csl] = \
                    d2[c, i0:i0 + IPF, :jp].reshape(-1)
                rf[4 * f + 2 * g + 1, csl] = \
                    d[c, i0:i0 + IPF, :jp].reshape(-1)
    types = np.where(mask, atom_types, 0).astype(np.int64)        # (B, N)
    h0 = emb_table[types]                                         # (B, N, H)
    h0T = np.ascontiguousarray(
        h0.transpose(2, 0, 1).reshape(H, B * N)).astype(np.float32)
    maskR = mask.astype(np.float32).reshape(1, B * N).astype(
        ml_dtypes.bfloat16)
    return [dict(rf64=rf, h0T=h0T, maskR=maskR, **shared)]


def _ensure_ntff_hook():
    """run_bass_kernel_spmd(trace=True) imports antenv.axon_hooks, which
    some agent images lack. If it's missing, register the equivalent hook
    from the boot module so a BASS_TRACE=1 run profiles instead of
    crashing. No-op when the real module exists."""
    import sys as _sys
    try:
        import antenv.axon_hooks  # noqa: F401
        return
    except ImportError:
        pass
    try:
        import types as _types
        import antenv  # noqa: F401
        import trn_agent_boot.trn_boot as _tb
        hook = _tb._ntff_profile_via_ctypes('/opt/axon/libaxon_pjrt.so')
        mod = _types.ModuleType('antenv.axon_hooks')
        mod.get_axon_ntff_profile_hook = lambda: hook
        mod.set_axon_ntff_profile_hook = lambda h: None
        _sys.modules['antenv.axon_hooks'] = mod
    except Exception:
        pass


def kernel(**inputs):
    from concourse.bass_utils import run_bass_kernel_spmd

    _ensure_ntff_hook()

    atom_types = np.asarray(inputs["atom_types"])
    frac_coords = np.asarray(inputs["frac_coords"], np.float32)
    lattice = np.asarray(inputs["lattice"], np.float32)
    mask = np.asarray(inputs["mask"]).astype(bool)
    emb_table = np.asarray(inputs["emb_table"], np.float32)
    edge_w = np.asarray(inputs["edge_w"], np.float32)
    edge_b = np.asarray(inputs["edge_b"], np.float32)
    node_w = np.asarray(inputs["node_w"], np.float32)
    node_b = np.asarray(inputs["node_b"], np.float32)
    mu_w = np.asarray(inputs["mu_w"], np.float32)
    mu_b = np.asarray(inputs["mu_b"], np.float32)
    var_w = np.asarray(inputs["var_w"], np.float32)
    var_b = np.asarray(inputs["var_b"], np.float32)

    lengths = mask.sum(1).astype(int)
    nc = _get_nc(lengths)
    in_maps = make_in_maps(atom_types, frac_coords, lattice, mask, emb_table,
                           edge_w, edge_b, node_w, node_b)
    res = run_bass_kernel_spmd(nc, in_maps, core_ids=[0])
    sum_h = np.ascontiguousarray(res.results[0]["sumh"].T)        # (B, H)
    n_valid = mask.sum(1).astype(np.float32)
    g = sum_h / (n_valid[:, None] + 1e-6)
    mu = (g @ mu_w + mu_b).astype(np.float32)
    log_var = (g @ var_w + var_b).astype(np.float32)
    return mu, log_var


# revision 46
# speedup vs baseline: 1.1700x; 1.0006x over previous
"""CrystalEncoder Trainium2 kernel (v4): all 8 crystals on ONE NeuronCore,
runtime-specialized to the ragged atom counts.

Why one core: in this axon environment each per-device NEFF dispatch carries
~1.2ms of launch overhead and the 8-device dispatch serializes them (~10ms
total — which is what the 9.25ms baseline number actually was). One dispatch
running all 8 crystals sequentially costs 1 launch + the compute.

Ragged specialization: lengths len_c (valid atoms) are in [N/2, N]. The
kernel is BUILT for the lengths observed in the inputs (cached per length
tuple; the build is pure emission, a few hundred ms):
  - j is trimmed to jp_c = len_c rounded up to even (host packs rf rows
    with jp pairs per i-row, so every on-device free dim scales by jp/N);
  - group-1 gate blocks are emitted only for valid i-rows (8-row
    granularity on the last block), and the node update only touches
    the first lp_c columns.
Invalid j inside jp contribute zero via h_j = 0 (padding embedding row);
invalid i inside lp are masked by maskF after the node update.

Per crystal (N=256, H=128, BINS=40, NL=2):
  1. rf64 slice: 16 fills x 4 rows (d^2/d x 2 i-groups), fill = 8 i-rows
     x jp pairs per group (f32r, host-computed).
  2. RBF exponents via K=64 matmuls (cE64 = 16 per-fill selector blocks),
     Exp bias -g*c_k^2 -> rbfT [128, 128*jp] bf16 (groups at partitions
     0/64, same free column = same (i_local, j) pair of each group).
  3. Per layer: gate matmuls (K=40 bf16, <=512-free, psum 8-i-row chunks);
     softplus = Exp then Ln(1+x) (one natural_log_exp table set); DVE 2x
     bf16 multiply by broadcast h_j + add-halves + reduce -> aggT; node
     update zT = node_w^T @ aggT + Silu + residual + mask.
  4. sum over atoms -> sumh column; one [H, 8] output DMA at the end.

Software pipelining: crystal c's layer-2 node update is deferred until
after crystal c+1's RBF stage, and layer-2's first two gate blocks are
produced before layer-1's node update, so ACT (the bottleneck engine)
never waits on DVE reduce tails. All element-wise work is on DVE (GpSimd
tensor ops are Q7 software at ~0.42 efficiency on real HW).

Sync discipline: this walrus build supports at most ONE semaphore wait per
instruction; _install_wait_splitter() splits multi-wait instructions with
same-engine NoOp carriers.
"""

import numpy as np
import ml_dtypes

B, N, H, LAT, NL, BINS = 8, 256, 128, 64, 2, 40
VMAX = 8.0
GAMMA = 1.0 / (VMAX / BINS) ** 2  # 25.0

G = 2                  # i-groups; bins at partition offsets 0 / 64
IPG = N // G           # 128 i-rows per group
NFILL = 16             # rf fills per crystal (8 i-rows per group each)
IPF = 8                # i-rows per fill per group
IPB = 32               # i-rows per gate block
IPC = 8                # i-rows per PSUM chunk
MMF = 512              # matmul free size (hard ISA limit)

_CACHE = {}


def _install_wait_splitter():
    """This walrus build supports at most ONE semaphore wait per ISA
    instruction. Split every multi-wait instruction by inserting same-engine
    NoOp carriers, each holding one of the waits, immediately before it."""
    import bass_rust
    import concourse.tile as tile
    from concourse import mybir

    if getattr(tile.TileContext, "_wait_split_installed", False):
        return
    orig = tile.TileContext._lower_ordered_insts
    counter = [0]

    def patched(self, ordered):
        for insts in ordered.values():
            newl = []
            for inst in insts:
                si = inst.sync_info
                ow = list(si.on_wait) if (si is not None and si.on_wait) else []
                if len(ow) > 1 and inst.engine != mybir.EngineType.Unassigned:
                    for w in ow[:-1]:
                        counter[0] += 1
                        nop = bass_rust.InstNoOp(
                            name=f"wsplit_{counter[0]}", ins=[], outs=[]
                        )
                        nop.engine = inst.engine
                        nop.sync_info = bass_rust.SyncInfo(
                            on_wait=[w], on_update=[]
                        )
                        newl.append(nop)
                    inst.sync_info = bass_rust.SyncInfo(
                        on_wait=[ow[-1]], on_update=list(si.on_update or [])
                    )
                newl.append(inst)
            insts[:] = newl
        return orig(self, ordered)

    tile.TileContext._lower_ordered_insts = patched

    def patched_dab(self, tick_clock, wait_clock):
        from concourse.vector_clock import ScopedClock

        probe = self.nc.sync.nop()
        wait_clock.add_sem_waits(
            probe.ins, ScopedClock({None: tick_clock.global_clock})
        )
        si = probe.ins.sync_info
        ow = list(si.on_wait) if (si is not None and si.on_wait) else []
        if len(ow) > 1:
            probe.ins.sync_info = bass_rust.SyncInfo(
                on_wait=[ow[0]], on_update=list(si.on_update or [])
            )
            for w in ow[1:]:
                n2 = self.nc.sync.nop()
                n2.ins.sync_info = bass_rust.SyncInfo(on_wait=[w], on_update=[])
        self.nc.sync.drain()
        self.nc.all_engine_barrier()
        popped = self.nc._tile_sem_poison_stack.pop()
        assert popped is self._sem_poison
        self.nc.clear_and_free_semaphores(list(self.sems.allocated().values()))
        self.nc.all_engine_barrier()

    tile.TileContext._drain_and_barrier = patched_dab
    tile.TileContext._wait_split_installed = True


def _crystal_geom(length):
    """Per-crystal specialization: (jp, blocks, lp).

    jp: j columns kept (even). blocks: [(g, i0_local, rows)] gate blocks —
    group 0 always 4x32 rows, group 1 in 32-row blocks plus an 8-granular
    remainder. lp = 128 + padded group-1 rows (i columns computed)."""
    length = int(length)
    jp = min(N, length + (length & 1))
    g1 = max(0, min(IPG, length - IPG))
    g1p = -(-g1 // IPC) * IPC
    blocks = [(0, i0, IPB) for i0 in range(0, IPG, IPB)]
    full, rem = divmod(g1p, IPB)
    for k in range(full):
        blocks.append((1, k * IPB, IPB))
    if rem:
        blocks.append((1, full * IPB, rem))
    lp = IPG + g1p
    return jp, blocks, lp


def _build_nc(lengths):
    import concourse.bass as bass
    import concourse.tile as tile
    from concourse import mybir

    _install_wait_splitter()

    F32 = mybir.dt.float32
    F32R = mybir.dt.float32r
    BF16 = mybir.dt.bfloat16
    AF = mybir.ActivationFunctionType
    X = mybir.AxisListType
    POOL = mybir.EngineType.Pool
    SP = mybir.EngineType.SP

    nc = bass.Bass("TRN2", target_bir_lowering=False, debug=False)

    def dep_nop(engine, aps):
        """Engine-local nop reading `aps`: pulls their producers' ticks into
        the engine's observed clock so later real instructions need at most
        one new semaphore wait."""
        nop = engine.nop(hint="dep").ins
        nop.ins = [engine.lower_ap(ap) for ap in aps]
        return nop

    NCR = len(lengths)
    rf_offs = [0]
    for c in range(NCR):
        jp_c, _, _ = _crystal_geom(lengths[c])
        rf_offs.append(rf_offs[-1] + IPF * jp_c)
    d_rf = nc.dram_tensor("rf64", [64, rf_offs[-1]], F32R,
                          kind="ExternalInput")
    d_cE = nc.dram_tensor("cE64", [64, NFILL * H], F32R, kind="ExternalInput")
    d_cbias = nc.dram_tensor("cbias", [H, 1], F32, kind="ExternalInput")
    d_ewR = nc.dram_tensor("ewR", [H, NL * H], BF16, kind="ExternalInput")
    d_ebT = nc.dram_tensor("ebT", [H, NL], F32, kind="ExternalInput")
    d_nwT = nc.dram_tensor("nwT", [H, NL * H], F32, kind="ExternalInput")
    d_nbT = nc.dram_tensor("nbT", [H, NL], F32, kind="ExternalInput")
    d_nbTn = nc.dram_tensor("nbTn", [H, NL], F32, kind="ExternalInput")
    d_h0T = nc.dram_tensor("h0T", [H, NCR * N], F32, kind="ExternalInput")
    d_maskR = nc.dram_tensor("maskR", [1, NCR * N], BF16, kind="ExternalInput")
    d_sumh = nc.dram_tensor("sumh", [H, NCR], F32, kind="ExternalOutput")

    with tile.TileContext(nc) as tc:
        with tc.tile_pool(name="consts", bufs=1) as consts, \
             tc.tile_pool(name="rfp", bufs=1) as rfp, \
             tc.tile_pool(name="lay", bufs=2) as lay, \
             tc.tile_pool(name="gxp", bufs=2) as gxp, \
             tc.tile_pool(name="gtp", bufs=2) as gtp, \
             tc.tile_pool(name="ppp", bufs=1) as ppp, \
             tc.tile_pool(name="tmp", bufs=1) as tmpp, \
             tc.tile_pool(name="ps", bufs=2, space="PSUM") as ps:
            kwp = dict(forced_dma_engine=POOL)
            kws = dict(forced_dma_engine=SP)
            # Pool queue stays short so crystal 0's rf DMA lands early;
            # everything bulky or late-needed goes via the SP queue.
            t_cE = consts.tile([64, NFILL * H], F32R, tag="cE")
            nc.sync.dma_start(out=t_cE[:, 0:H], in_=d_cE[:, 0:H])
            nc.sync.dma_start(out=t_cE[:, H:], in_=d_cE[:, H:])
            t_cbias = consts.tile_from(d_cbias[:], **kwp)
            t_ebT = consts.tile_from(d_ebT[:], **kwp)
            t_nbT = consts.tile_from(d_nbT[:], **kwp)
            t_nbTn = consts.tile_from(d_nbTn[:], **kwp)
            t_ewR = consts.tile_from(d_ewR[:], **kws)
            t_nwT = consts.tile_from(d_nwT[:], **kws)
            t_h = consts.tile_from(d_h0T[:], **kws)
            t_maskR = consts.tile_from(d_maskR[:], **kws)
            t_ones = consts.tile([1, H], BF16, tag="ones")
            t_maskF = consts.tile([H, NCR * N], BF16, tag="maskF")

            rbfT = consts.tile([H, IPG * N], BF16)
            sumh = consts.tile([H, NCR], F32, tag="sumh")

            dep_nop(nc.tensor, [t_cE[:], t_ewR[:], t_nwT[:], t_maskR[:]])
            dep_nop(nc.scalar, [t_cbias[:], t_ebT[:], t_nbT[:], t_nbTn[:]])
            dep_nop(nc.vector, [t_h[:]])

            nc.vector.memset(t_ones[:], 1.0)
            dep_nop(nc.tensor, [t_ones[:]])

            def expand_mask():
                """Expand the mask row to all H partitions: ones^T @ maskR
                via K=1 matmuls, copied out of PSUM on DVE. Emitted after
                the first crystal's RBF stage so it never delays the first
                exponent matmuls (maskR is the last const DMA to land);
                it is only needed at the first node update."""
                for q in range(NCR * N // (IPC * N)):
                    mp = ps.tile([H, IPC * N], F32, tag="ps")
                    for s in range(IPC * N // MMF):
                        f0 = q * IPC * N + s * MMF
                        nc.tensor.matmul(
                            mp[:, s * MMF:(s + 1) * MMF], t_ones[:],
                            t_maskR[:, f0:f0 + MMF], start=True, stop=True,
                        )
                    nc.vector.tensor_copy(
                        t_maskF[:, q * IPC * N:(q + 1) * IPC * N], mp[:])

            def stage2(c, jp):
                """RBF table build for crystal c: rf DMA, K=64 exponent
                matmuls per fill, Exp -> rbfT[:, :128*jp]."""
                fw = IPF * jp                       # free width per fill
                rf = rfp.tile([64, IPF * N], F32R, tag="rf")
                if c == order[0]:
                    # first crystal: land the first matmul's rhs columns
                    # ahead of the bulk so PE starts ~3us earlier
                    nc.gpsimd.dma_start(
                        out=rf[:, :MMF],
                        in_=d_rf[:, rf_offs[c]:rf_offs[c] + MMF])
                    nc.gpsimd.dma_start(
                        out=rf[:, MMF:fw],
                        in_=d_rf[:, rf_offs[c] + MMF:rf_offs[c] + fw])
                else:
                    nc.gpsimd.dma_start(
                        out=rf[:, :fw],
                        in_=d_rf[:, rf_offs[c]:rf_offs[c] + fw])
                dep_nop(nc.tensor, [rf[:]])
                for f in range(NFILL):
                    e = ps.tile([H, IPC * N], F32, tag="ps")
                    for s in range(-(-fw // MMF)):
                        w = min(MMF, fw - s * MMF)
                        nc.tensor.matmul(
                            e[:, s * MMF:s * MMF + w],
                            t_cE[:, f * H:(f + 1) * H],
                            rf[:, s * MMF:s * MMF + w],
                            start=True, stop=True,
                        )
                    nc.scalar.activation(
                        rbfT[:, f * fw:(f + 1) * fw], e[:, :fw], AF.Exp,
                        bias=t_cbias[:],
                    )

            def gate_produce(l, blk, jp):
                """Gate matmuls + Exp + Ln for one (g, i0_local, rows)
                block."""
                g, i0l, rows = blk
                bw = rows * jp                      # block free width
                lf = i0l * jp
                rp = min(rows, (IPC * N) // jp)     # i-rows per PSUM chunk
                gx = gxp.tile([H, IPB * N], BF16, tag="gx")
                done = 0
                while done < rows:
                    r = min(rp, rows - done)
                    cw = r * jp
                    gp = ps.tile([H, IPC * N], F32, tag="ps")
                    for s in range(-(-cw // MMF)):
                        w = min(MMF, cw - s * MMF)
                        f0 = lf + done * jp + s * MMF
                        nc.tensor.matmul(
                            gp[:, s * MMF:s * MMF + w],
                            t_ewR[64 * g:64 * g + BINS, l * H:(l + 1) * H],
                            rbfT[64 * g:64 * g + BINS, f0:f0 + w],
                            start=True, stop=True,
                        )
                    nc.scalar.activation(
                        gx[:, done * jp:done * jp + cw], gp[:, :cw], AF.Exp,
                        bias=t_ebT[:, l:l + 1],
                    )
                    done += r
                gt = gtp.tile([H, IPB * N], BF16, tag="gt")
                nc.scalar.activation(gt[:, :bw], gx[:, :bw], AF.Ln, bias=1.0)
                return gt

            def gate_consume(gt, blk, jp, hmr, aggT, split=False):
                """DVE: pp = gt * h_j; add j-halves; reduce -> aggT cols."""
                g, i0l, rows = blk
                i0 = g * IPG + i0l
                subs = (rows // IPC) if split else 1
                rw = rows // subs
                w = rw * jp
                pp = ppp.tile([H, IPB * N], BF16, tag="pp")
                tm = tmpp.tile([H, IPB * N // 2], BF16, tag="tm")
                for s in range(subs):
                    sl_ = slice(s * w, (s + 1) * w)
                    ppv = pp[:, sl_].rearrange("p (r c) -> p r c", c=jp)
                    nc.vector.tensor_mul(
                        ppv,
                        gt[:, sl_].rearrange("p (r c) -> p r c", c=jp),
                        hmr[:, None, :jp].broadcast_to([H, rw, jp]),
                    )
                    tmv = tm[:, s * w // 2:(s + 1) * w // 2].rearrange(
                        "p (r c) -> p r c", c=jp // 2)
                    nc.vector.tensor_add(
                        tmv, ppv[:, :, 0:jp // 2], ppv[:, :, jp // 2:jp])
                    nc.vector.reduce_sum(
                        out=aggT[:, i0 + s * rw:i0 + (s + 1) * rw],
                        in_=tmv, axis=X.X,
                    )

            def node_update(c, l, aggT, lp):
                """zT = node_w^T @ aggT; h += silu(zT + b); h *= mask.
                silu(z) = z * exp(-ln(1 + exp(-z))) uses only the
                natural_log_exp table set — no ACT table switches.
                Only the first lp columns are computed columns."""
                hsl = slice(c * N, c * N + lp)
                dep_nop(nc.tensor, [aggT[:]])
                zp = ps.tile([H, IPC * N], F32, tag="ps")
                nc.tensor.matmul(
                    zp[:, :lp], t_nwT[:, l * H:(l + 1) * H], aggT[:, :lp],
                    start=True, stop=True,
                )
                # z = zp + node_b (fold bias into the first Exp's scale
                # trick is not possible: bias applies pre-function), so
                # add it on DVE first.
                # clamp first (only dep: zp) so the ACT chain starts after
                # ONE DVE op; the unclamped z for the final multiply is
                # computed in parallel with the ACT chain. silu(z<-30) ~ 0.
                ztc = lay.tile([H, N], F32, tag="ztc")
                nc.vector.tensor_scalar_max(ztc[:, :lp], zp[:, :lp], -30.0)
                zt = lay.tile([H, N], F32, tag="zt")
                nc.vector.tensor_scalar_add(
                    zt[:, :lp], zp[:, :lp], t_nbT[:, l:l + 1])
                u = lay.tile([H, N], F32, tag="sgu")
                nc.scalar.activation(u[:, :lp], ztc[:, :lp], AF.Exp,
                                     scale=-1.0, bias=t_nbTn[:, l:l + 1])
                w = lay.tile([H, N], F32, tag="sgw")
                nc.scalar.activation(w[:, :lp], u[:, :lp], AF.Ln, bias=1.0)
                sg = lay.tile([H, N], F32, tag="sgs")
                nc.scalar.activation(sg[:, :lp], w[:, :lp], AF.Exp,
                                     scale=-1.0)
                sl = lay.tile([H, N], F32, tag="sil")
                nc.vector.tensor_mul(sl[:, :lp], zt[:, :lp], sg[:, :lp])
                h2 = lay.tile([H, N], F32, tag="h2")
                nc.vector.tensor_add(h2[:, :lp], t_h[:, hsl], sl[:, :lp])
                nc.vector.tensor_mul(t_h[:, hsl], h2[:, :lp],
                                     t_maskF[:, hsl])

            deferred = None  # (c, aggT2, lp) awaiting layer-2 node update

            def finish_crystal(dfr):
                c, aggT2, lp = dfr
                node_update(c, 1, aggT2, lp)
                nc.vector.reduce_sum(
                    out=sumh[:, c:c + 1], in_=t_h[:, c * N:(c + 1) * N],
                    axis=X.X,
                )

            # longest crystals first: the final (un-hideable) reduce tail
            # then belongs to the shortest crystal
            order = sorted(range(NCR), key=lambda c: -int(lengths[c]))
            first = True
            for c in order:
                jp, blocks, lp = _crystal_geom(lengths[c])
                nblk = len(blocks)
                stage2(c, jp)
                if first:
                    expand_mask()
                    first = False
                if deferred is not None:
                    finish_crystal(deferred)
                    deferred = None
                hsl = slice(c * N, (c + 1) * N)
                hmr1 = lay.tile([H, N], BF16, tag="hmr1")
                nc.vector.tensor_copy(hmr1[:], t_h[:, hsl])
                # layer 1
                aggT1 = lay.tile([H, N], F32, tag="agg1")
                for b in range(nblk):
                    gt = gate_produce(0, blocks[b], jp)
                    gate_consume(gt, blocks[b], jp, hmr1, aggT1,
                                 split=(b == nblk - 1))
                # layer 2: produce first two blocks before layer-1 node
                # update so ACT stays busy over the layer-1 reduce tail
                aggT2 = lay.tile([H, N], F32, tag="agg2")
                gt20 = gate_produce(1, blocks[0], jp)
                gt21 = gate_produce(1, blocks[1], jp)
                node_update(c, 0, aggT1, lp)
                hmr2 = lay.tile([H, N], BF16, tag="hmr2")
                nc.vector.tensor_copy(hmr2[:], t_h[:, hsl])
                gate_consume(gt20, blocks[0], jp, hmr2, aggT2)
                gate_consume(gt21, blocks[1], jp, hmr2, aggT2)
                for b in range(2, nblk):
                    gt = gate_produce(1, blocks[b], jp)
                    gate_consume(gt, blocks[b], jp, hmr2, aggT2,
                                 split=(b == nblk - 1))
                deferred = (c, aggT2, lp)

            finish_crystal(deferred)
            nc.gpsimd.dma_start(out=d_sumh[:], in_=sumh[:])

    return nc


def _get_nc(lengths):
    key = tuple(int(x) for x in lengths)
    if key not in _CACHE:
        _CACHE[key] = _build_nc(key)
    return _CACHE[key]


def _shared_inputs(edge_w, edge_b, node_w, node_b):
    centers = np.linspace(0.0, VMAX, BINS).astype(np.float64)
    # cE64: 16 per-fill selector blocks. Fill f uses rf rows 4f..4f+3:
    # row 4f+2g+0 = d^2 of group g, row 4f+2g+1 = d of group g.
    cE = np.zeros((64, NFILL * H), np.float32)
    for f in range(NFILL):
        for g in range(G):
            col0 = f * H + 64 * g
            cE[4 * f + 2 * g + 0, col0:col0 + BINS] = -GAMMA
            cE[4 * f + 2 * g + 1, col0:col0 + BINS] = 2.0 * GAMMA * centers
    cbias = np.zeros((H, 1), np.float32)
    ewR = np.zeros((H, NL * H), np.float32)
    for g in range(G):
        cbias[64 * g:64 * g + BINS, 0] = -GAMMA * centers * centers
        for l in range(NL):
            ewR[64 * g:64 * g + BINS, l * H:(l + 1) * H] = edge_w[l]
    ewR = ewR.astype(ml_dtypes.bfloat16)
    ebT = np.ascontiguousarray(edge_b.T).astype(np.float32)      # [H, NL]
    nwT = np.concatenate([node_w[l] for l in range(NL)], axis=1)
    nwT = np.ascontiguousarray(nwT).astype(np.float32)           # [H, NL*H]
    nbT = np.ascontiguousarray(node_b.T).astype(np.float32)      # [H, NL]
    return dict(cE64=cE, cbias=cbias, ewR=ewR, ebT=ebT, nwT=nwT, nbT=nbT,
                nbTn=-nbT)


def make_in_maps(atom_types, frac_coords, lattice, mask, emb_table,
                 edge_w, edge_b, node_w, node_b):
    shared = _shared_inputs(edge_w, edge_b, node_w, node_b)
    lengths = mask.sum(1).astype(int)
    cart = np.einsum('bnd,bde->bne', frac_coords, lattice).astype(np.float32)
    nsq = (cart * cart).sum(-1)                                   # (B, N)
    d2 = (nsq[:, :, None] + nsq[:, None, :]
          - 2.0 * np.einsum('bid,bjd->bij', cart, cart))
    d2 = np.maximum(d2, 0.0).astype(np.float32) + np.float32(1e-6)
    d = np.sqrt(d2)
    # rf64 [64, sum(8*jp_c)]: crystal c at its packed offset; fill f rows
    # 4f+2g+{0,1} = (d^2, d) of group g, i-rows [8f, 8f+8), j < jp_c,
    # row-major over (i, j).
    offs = [0]
    jps = []
    for c in range(B):
        jp, _, _ = _crystal_geom(lengths[c])
        jps.append(jp)
        offs.append(offs[-1] + IPF * jp)
    rf = np.zeros((64, offs[-1]), np.float32)
    for c in range(B):
        jp = jps[c]
        fw = IPF * jp
        csl = slice(offs[c], offs[c] + fw)
        for f in range(NFILL):
            for g in range(G):
                i0 = g * IPG + f * IPF
                rf[4 * f + 2 * g + 0, csl] = \
                    d2[c, i0:i0 + IPF, :jp].reshape(-1)
                rf[4 * f + 2 * g + 1, csl] = \
                    d[c, i0:i0 + IPF, :jp].reshape(-1)
    types = np.where(mask, atom_types, 0).astype(np.int64)        # (B, N)
    h0 = emb_table[types]                                         # (B, N, H)
    h0T = np.ascontiguousarray(
        h0.transpose(2, 0, 1).reshape(H, B * N)).astype(np.float32)
    maskR = mask.astype(np.float32).reshape(1, B * N).astype(
        ml_dtypes.bfloat16)
    return [dict(rf64=rf, h0T=h0T, maskR=maskR, **shared)]


def _ensure_ntff_hook():
    """run_bass_kernel_spmd(trace=True) imports antenv.axon_hooks, which
    some agent images lack. If it's missing, register the equivalent hook
    from the boot module so a BASS_TRACE=1 run profiles instead of
    crashing. No-op when the real module exists."""
    import sys as _sys
    try:
        import antenv.axon_hooks  # noqa: F401
        return
    except ImportError:
        pass
    try:
        import types as _types
        import antenv  # noqa: F401
        import trn_agent_boot.trn_boot as _tb
        hook = _tb._ntff_profile_via_ctypes('/opt/axon/libaxon_pjrt.so')
        mod = _types.ModuleType('antenv.axon_hooks')
        mod.get_axon_ntff_profile_hook = lambda: hook
        mod.set_axon_ntff_profile_hook = lambda h: None
        _sys.modules['antenv.axon_hooks'] = mod
    except Exception:
        pass


def kernel(**inputs):
    from concourse.bass_utils import run_bass_kernel_spmd

    _ensure_ntff_hook()

    atom_types = np.asarray(inputs["atom_types"])
    frac_coords = np.asarray(inputs["frac_coords"], np.float32)
    lattice = np.asarray(inputs["lattice"], np.float32)
    mask = np.asarray(inputs["mask"]).astype(bool)
    emb_table = np.asarray(inputs["emb_table"], np.float32)
    edge_w = np.asarray(inputs["edge_w"], np.float32)
    edge_b = np.asarray(inputs["edge_b"], np.float32)
    node_w = np.asarray(inputs["node_w"], np.float32)
    node_b = np.asarray(inputs["node_b"], np.float32)
    mu_w = np.asarray(inputs["mu_w"], np.float32)
    mu_b = np.asarray(inputs["mu_b"], np.float32)
    var_w = np.asarray(inputs["var_w"], np.float32)
    var_b = np.asarray(inputs["var_b"], np.float32)

    lengths = mask.sum(1).astype(int)
    nc = _get_nc(lengths)
    in_maps = make_in_maps(atom_types, frac_coords, lattice, mask, emb_table,
                           edge_w, edge_b, node_w, node_b)
    res = run_bass_kernel_spmd(nc, in_maps, core_ids=[0])
    sum_h = np.ascontiguousarray(res.results[0]["sumh"].T)        # (B, H)
    n_valid = mask.sum(1).astype(np.float32)
    g = sum_h / (n_valid[:, None] + 1e-6)
    mu = (g @ mu_w + mu_b).astype(np.float32)
    log_var = (g @ var_w + var_b).astype(np.float32)
    return mu, log_var


# revision 47
# speedup vs baseline: 1.1753x; 1.0046x over previous
"""CrystalEncoder Trainium2 kernel (v4): all 8 crystals on ONE NeuronCore,
runtime-specialized to the ragged atom counts.

Why one core: in this axon environment each per-device NEFF dispatch carries
~1.2ms of launch overhead and the 8-device dispatch serializes them (~10ms
total — which is what the 9.25ms baseline number actually was). One dispatch
running all 8 crystals sequentially costs 1 launch + the compute.

Ragged specialization: lengths len_c (valid atoms) are in [N/2, N]. The
kernel is BUILT for the lengths observed in the inputs (cached per length
tuple; the build is pure emission, a few hundred ms):
  - j is trimmed to jp_c = len_c rounded up to even (host packs rf rows
    with jp pairs per i-row, so every on-device free dim scales by jp/N);
  - group-1 gate blocks are emitted only for valid i-rows (8-row
    granularity on the last block), and the node update only touches
    the first lp_c columns.
Invalid j inside jp contribute zero via h_j = 0 (padding embedding row);
invalid i inside lp are masked by maskF after the node update.

Per crystal (N=256, H=128, BINS=40, NL=2):
  1. rf64 slice: 16 fills x 4 rows (d^2/d x 2 i-groups), fill = 8 i-rows
     x jp pairs per group (f32r, host-computed).
  2. RBF exponents via K=64 matmuls (cE64 = 16 per-fill selector blocks),
     Exp bias -g*c_k^2 -> rbfT [128, 128*jp] bf16 (groups at partitions
     0/64, same free column = same (i_local, j) pair of each group).
  3. Per layer: gate matmuls (K=40 bf16, <=512-free, psum 8-i-row chunks);
     softplus = Exp then Ln(1+x) (one natural_log_exp table set); DVE 2x
     bf16 multiply by broadcast h_j + add-halves + reduce -> aggT; node
     update zT = node_w^T @ aggT + Silu + residual + mask.
  4. sum over atoms -> sumh column; one [H, 8] output DMA at the end.

Software pipelining: crystal c's layer-2 node update is deferred until
after crystal c+1's RBF stage, and layer-2's first two gate blocks are
produced before layer-1's node update, so ACT (the bottleneck engine)
never waits on DVE reduce tails. All element-wise work is on DVE (GpSimd
tensor ops are Q7 software at ~0.42 efficiency on real HW).

Sync discipline: this walrus build supports at most ONE semaphore wait per
instruction; _install_wait_splitter() splits multi-wait instructions with
same-engine NoOp carriers.
"""

import numpy as np
import ml_dtypes

B, N, H, LAT, NL, BINS = 8, 256, 128, 64, 2, 40
VMAX = 8.0
GAMMA = 1.0 / (VMAX / BINS) ** 2  # 25.0

G = 2                  # i-groups; bins at partition offsets 0 / 64
IPG = N // G           # 128 i-rows per group
NFILL = 16             # rf fills per crystal (8 i-rows per group each)
IPF = 8                # i-rows per fill per group
IPB = 32               # i-rows per gate block
IPC = 8                # i-rows per PSUM chunk
MMF = 512              # matmul free size (hard ISA limit)

_CACHE = {}


def _install_wait_splitter():
    """This walrus build supports at most ONE semaphore wait per ISA
    instruction. Split every multi-wait instruction by inserting same-engine
    NoOp carriers, each holding one of the waits, immediately before it."""
    import bass_rust
    import concourse.tile as tile
    from concourse import mybir

    if getattr(tile.TileContext, "_wait_split_installed", False):
        return
    orig = tile.TileContext._lower_ordered_insts
    counter = [0]

    def patched(self, ordered):
        for insts in ordered.values():
            newl = []
            for inst in insts:
                si = inst.sync_info
                ow = list(si.on_wait) if (si is not None and si.on_wait) else []
                if len(ow) > 1 and inst.engine != mybir.EngineType.Unassigned:
                    for w in ow[:-1]:
                        counter[0] += 1
                        nop = bass_rust.InstNoOp(
                            name=f"wsplit_{counter[0]}", ins=[], outs=[]
                        )
                        nop.engine = inst.engine
                        nop.sync_info = bass_rust.SyncInfo(
                            on_wait=[w], on_update=[]
                        )
                        newl.append(nop)
                    inst.sync_info = bass_rust.SyncInfo(
                        on_wait=[ow[-1]], on_update=list(si.on_update or [])
                    )
                newl.append(inst)
            insts[:] = newl
        return orig(self, ordered)

    tile.TileContext._lower_ordered_insts = patched

    def patched_dab(self, tick_clock, wait_clock):
        from concourse.vector_clock import ScopedClock

        probe = self.nc.sync.nop()
        wait_clock.add_sem_waits(
            probe.ins, ScopedClock({None: tick_clock.global_clock})
        )
        si = probe.ins.sync_info
        ow = list(si.on_wait) if (si is not None and si.on_wait) else []
        if len(ow) > 1:
            probe.ins.sync_info = bass_rust.SyncInfo(
                on_wait=[ow[0]], on_update=list(si.on_update or [])
            )
            for w in ow[1:]:
                n2 = self.nc.sync.nop()
                n2.ins.sync_info = bass_rust.SyncInfo(on_wait=[w], on_update=[])
        self.nc.sync.drain()
        self.nc.all_engine_barrier()
        popped = self.nc._tile_sem_poison_stack.pop()
        assert popped is self._sem_poison
        self.nc.clear_and_free_semaphores(list(self.sems.allocated().values()))
        self.nc.all_engine_barrier()

    tile.TileContext._drain_and_barrier = patched_dab
    tile.TileContext._wait_split_installed = True


def _crystal_geom(length):
    """Per-crystal specialization: (jp, blocks, lp).

    jp: j columns kept (even). blocks: [(g, i0_local, rows)] gate blocks —
    group 0 always 4x32 rows, group 1 in 32-row blocks plus an 8-granular
    remainder. lp = 128 + padded group-1 rows (i columns computed)."""
    length = int(length)
    jp = min(N, length + (length & 1))
    g1 = max(0, min(IPG, length - IPG))
    g1p = -(-g1 // IPC) * IPC
    blocks = [(0, i0, IPB) for i0 in range(0, IPG, IPB)]
    full, rem = divmod(g1p, IPB)
    for k in range(full):
        blocks.append((1, k * IPB, IPB))
    if rem:
        blocks.append((1, full * IPB, rem))
    lp = IPG + g1p
    return jp, blocks, lp


def _build_nc(lengths):
    import concourse.bass as bass
    import concourse.tile as tile
    from concourse import mybir

    _install_wait_splitter()

    F32 = mybir.dt.float32
    F32R = mybir.dt.float32r
    BF16 = mybir.dt.bfloat16
    AF = mybir.ActivationFunctionType
    X = mybir.AxisListType
    POOL = mybir.EngineType.Pool
    SP = mybir.EngineType.SP

    nc = bass.Bass("TRN2", target_bir_lowering=False, debug=False)

    def dep_nop(engine, aps):
        """Engine-local nop reading `aps`: pulls their producers' ticks into
        the engine's observed clock so later real instructions need at most
        one new semaphore wait."""
        nop = engine.nop(hint="dep").ins
        nop.ins = [engine.lower_ap(ap) for ap in aps]
        return nop

    NCR = len(lengths)
    rf_offs = [0]
    for c in range(NCR):
        jp_c, _, _ = _crystal_geom(lengths[c])
        rf_offs.append(rf_offs[-1] + IPF * jp_c)
    d_rf = nc.dram_tensor("rf64", [64, rf_offs[-1]], F32R,
                          kind="ExternalInput")
    d_cE = nc.dram_tensor("cE64", [64, NFILL * H], F32R, kind="ExternalInput")
    d_cbias = nc.dram_tensor("cbias", [H, 1], F32, kind="ExternalInput")
    d_ewR = nc.dram_tensor("ewR", [H, NL * H], BF16, kind="ExternalInput")
    d_ebT = nc.dram_tensor("ebT", [H, NL], F32, kind="ExternalInput")
    d_nwT = nc.dram_tensor("nwT", [H, NL * H], F32, kind="ExternalInput")
    d_nbT = nc.dram_tensor("nbT", [H, NL], F32, kind="ExternalInput")
    d_nbTn = nc.dram_tensor("nbTn", [H, NL], F32, kind="ExternalInput")
    d_h0T = nc.dram_tensor("h0T", [H, NCR * N], F32, kind="ExternalInput")
    d_maskR = nc.dram_tensor("maskR", [1, NCR * N], BF16, kind="ExternalInput")
    d_sumh = nc.dram_tensor("sumh", [H, NCR], F32, kind="ExternalOutput")

    with tile.TileContext(nc) as tc:
        with tc.tile_pool(name="consts", bufs=1) as consts, \
             tc.tile_pool(name="rfp", bufs=1) as rfp, \
             tc.tile_pool(name="lay", bufs=2) as lay, \
             tc.tile_pool(name="gxp", bufs=2) as gxp, \
             tc.tile_pool(name="gtp", bufs=2) as gtp, \
             tc.tile_pool(name="ppp", bufs=1) as ppp, \
             tc.tile_pool(name="tmp", bufs=1) as tmpp, \
             tc.tile_pool(name="ps", bufs=2, space="PSUM") as ps:
            kwp = dict(forced_dma_engine=POOL)
            kws = dict(forced_dma_engine=SP)
            # Pool queue stays short so crystal 0's rf DMA lands early;
            # everything bulky or late-needed goes via the SP queue.
            t_cE = consts.tile([64, NFILL * H], F32R, tag="cE")
            nc.sync.dma_start(out=t_cE[:, 0:H], in_=d_cE[:, 0:H])
            nc.sync.dma_start(out=t_cE[:, H:], in_=d_cE[:, H:])
            t_cbias = consts.tile_from(d_cbias[:], **kwp)
            t_ebT = consts.tile_from(d_ebT[:], **kwp)
            t_nbT = consts.tile_from(d_nbT[:], **kwp)
            t_nbTn = consts.tile_from(d_nbTn[:], **kwp)
            t_ewR = consts.tile_from(d_ewR[:], **kws)
            t_nwT = consts.tile_from(d_nwT[:], **kws)
            t_h = consts.tile_from(d_h0T[:], **kws)
            t_maskR = consts.tile_from(d_maskR[:], **kws)
            t_ones = consts.tile([1, H], BF16, tag="ones")
            t_maskF = consts.tile([H, NCR * N], BF16, tag="maskF")

            rbfT = consts.tile([H, IPG * N], BF16)
            sumh = consts.tile([H, NCR], F32, tag="sumh")

            dep_nop(nc.tensor, [t_cE[:], t_ewR[:], t_nwT[:], t_maskR[:]])
            dep_nop(nc.scalar, [t_cbias[:], t_ebT[:], t_nbT[:], t_nbTn[:]])
            dep_nop(nc.vector, [t_h[:]])

            nc.vector.memset(t_ones[:], 1.0)
            dep_nop(nc.tensor, [t_ones[:]])

            def expand_mask():
                """Expand the mask row to all H partitions: ones^T @ maskR
                via K=1 matmuls, copied out of PSUM on DVE. Emitted after
                the first crystal's RBF stage so it never delays the first
                exponent matmuls (maskR is the last const DMA to land);
                it is only needed at the first node update."""
                for q in range(NCR * N // (IPC * N)):
                    mp = ps.tile([H, IPC * N], F32, tag="ps")
                    for s in range(IPC * N // MMF):
                        f0 = q * IPC * N + s * MMF
                        nc.tensor.matmul(
                            mp[:, s * MMF:(s + 1) * MMF], t_ones[:],
                            t_maskR[:, f0:f0 + MMF], start=True, stop=True,
                        )
                    nc.vector.tensor_copy(
                        t_maskF[:, q * IPC * N:(q + 1) * IPC * N], mp[:])

            def stage2(c, jp):
                """RBF table build for crystal c: rf DMA, K=64 exponent
                matmuls per fill, Exp -> rbfT[:, :128*jp]."""
                fw = IPF * jp                       # free width per fill
                rf = rfp.tile([64, IPF * N], F32R, tag="rf")
                if c == order[0]:
                    # first crystal: land the first matmul's rhs columns
                    # ahead of the bulk so PE starts ~3us earlier
                    nc.gpsimd.dma_start(
                        out=rf[:, :MMF],
                        in_=d_rf[:, rf_offs[c]:rf_offs[c] + MMF])
                    nc.gpsimd.dma_start(
                        out=rf[:, MMF:fw],
                        in_=d_rf[:, rf_offs[c] + MMF:rf_offs[c] + fw])
                else:
                    nc.gpsimd.dma_start(
                        out=rf[:, :fw],
                        in_=d_rf[:, rf_offs[c]:rf_offs[c] + fw])
                dep_nop(nc.tensor, [rf[:]])
                for f in range(NFILL):
                    e = ps.tile([H, IPC * N], F32, tag="ps")
                    for s in range(-(-fw // MMF)):
                        w = min(MMF, fw - s * MMF)
                        nc.tensor.matmul(
                            e[:, s * MMF:s * MMF + w],
                            t_cE[:, f * H:(f + 1) * H],
                            rf[:, s * MMF:s * MMF + w],
                            start=True, stop=True,
                        )
                    nc.scalar.activation(
                        rbfT[:, f * fw:(f + 1) * fw], e[:, :fw], AF.Exp,
                        bias=t_cbias[:],
                    )

            def gate_produce(l, blk, jp):
                """Gate matmuls + Exp + Ln for one (g, i0_local, rows)
                block."""
                g, i0l, rows = blk
                bw = rows * jp                      # block free width
                lf = i0l * jp
                rp = min(rows, (IPC * N) // jp)     # i-rows per PSUM chunk
                gx = gxp.tile([H, IPB * N], BF16, tag="gx")
                done = 0
                while done < rows:
                    r = min(rp, rows - done)
                    cw = r * jp
                    gp = ps.tile([H, IPC * N], F32, tag="ps")
                    for s in range(-(-cw // MMF)):
                        w = min(MMF, cw - s * MMF)
                        f0 = lf + done * jp + s * MMF
                        nc.tensor.matmul(
                            gp[:, s * MMF:s * MMF + w],
                            t_ewR[64 * g:64 * g + BINS, l * H:(l + 1) * H],
                            rbfT[64 * g:64 * g + BINS, f0:f0 + w],
                            start=True, stop=True,
                        )
                    nc.scalar.activation(
                        gx[:, done * jp:done * jp + cw], gp[:, :cw], AF.Exp,
                        bias=t_ebT[:, l:l + 1],
                    )
                    done += r
                gt = gtp.tile([H, IPB * N], BF16, tag="gt")
                nc.scalar.activation(gt[:, :bw], gx[:, :bw], AF.Ln, bias=1.0)
                return gt

            def gate_consume(gt, blk, jp, hmr, aggT, split=False):
                """DVE: pp = gt * h_j; add j-halves; reduce -> aggT cols."""
                g, i0l, rows = blk
                i0 = g * IPG + i0l
                subs = (rows // IPC) if split else 1
                rw = rows // subs
                w = rw * jp
                pp = ppp.tile([H, IPB * N], BF16, tag="pp")
                tm = tmpp.tile([H, IPB * N // 2], BF16, tag="tm")
                for s in range(subs):
                    sl_ = slice(s * w, (s + 1) * w)
                    ppv = pp[:, sl_].rearrange("p (r c) -> p r c", c=jp)
                    nc.vector.tensor_mul(
                        ppv,
                        gt[:, sl_].rearrange("p (r c) -> p r c", c=jp),
                        hmr[:, None, :jp].broadcast_to([H, rw, jp]),
                    )
                    tmv = tm[:, s * w // 2:(s + 1) * w // 2].rearrange(
                        "p (r c) -> p r c", c=jp // 2)
                    nc.vector.tensor_add(
                        tmv, ppv[:, :, 0:jp // 2], ppv[:, :, jp // 2:jp])
                    nc.vector.reduce_sum(
                        out=aggT[:, i0 + s * rw:i0 + (s + 1) * rw],
                        in_=tmv, axis=X.X,
                    )

            def node_update(c, l, aggT, lp):
                """zT = node_w^T @ aggT; h += silu(zT + b); h *= mask.
                silu(z) = z * exp(-ln(1 + exp(-z))) uses only the
                natural_log_exp table set — no ACT table switches.
                Only the first lp columns are computed columns."""
                hsl = slice(c * N, c * N + lp)
                dep_nop(nc.tensor, [aggT[:]])
                zp = ps.tile([H, IPC * N], F32, tag="ps")
                nc.tensor.matmul(
                    zp[:, :lp], t_nwT[:, l * H:(l + 1) * H], aggT[:, :lp],
                    start=True, stop=True,
                )
                # z = zp + node_b (fold bias into the first Exp's scale
                # trick is not possible: bias applies pre-function), so
                # add it on DVE first.
                # clamp first (only dep: zp) so the ACT chain starts after
                # ONE DVE op; the unclamped z for the final multiply is
                # computed in parallel with the ACT chain. silu(z<-30) ~ 0.
                ztc = lay.tile([H, N], F32, tag="ztc")
                nc.vector.tensor_scalar_max(ztc[:, :lp], zp[:, :lp], -30.0)
                zt = lay.tile([H, N], F32, tag="zt")
                nc.vector.tensor_scalar_add(
                    zt[:, :lp], zp[:, :lp], t_nbT[:, l:l + 1])
                u = lay.tile([H, N], F32, tag="sgu")
                nc.scalar.activation(u[:, :lp], ztc[:, :lp], AF.Exp,
                                     scale=-1.0, bias=t_nbTn[:, l:l + 1])
                w = lay.tile([H, N], F32, tag="sgw")
                nc.scalar.activation(w[:, :lp], u[:, :lp], AF.Ln, bias=1.0)
                sg = lay.tile([H, N], F32, tag="sgs")
                nc.scalar.activation(sg[:, :lp], w[:, :lp], AF.Exp,
                                     scale=-1.0)
                sl = lay.tile([H, N], F32, tag="sil")
                nc.vector.tensor_mul(sl[:, :lp], zt[:, :lp], sg[:, :lp])
                h2 = lay.tile([H, N], F32, tag="h2")
                nc.vector.tensor_add(h2[:, :lp], t_h[:, hsl], sl[:, :lp])
                nc.vector.tensor_mul(t_h[:, hsl], h2[:, :lp],
                                     t_maskF[:, hsl])

            deferred = None  # (c, aggT2, lp) awaiting layer-2 node update

            def finish_crystal(dfr):
                c, aggT2, lp = dfr
                node_update(c, 1, aggT2, lp)
                nc.vector.reduce_sum(
                    out=sumh[:, c:c + 1], in_=t_h[:, c * N:(c + 1) * N],
                    axis=X.X,
                )

            # longest crystals first: the final (un-hideable) reduce tail
            # then belongs to the shortest crystal
            order = sorted(range(NCR), key=lambda c: -int(lengths[c]))
            first = True
            for c in order:
                jp, blocks, lp = _crystal_geom(lengths[c])
                nblk = len(blocks)
                stage2(c, jp)
                if first:
                    expand_mask()
                    first = False
                hsl = slice(c * N, (c + 1) * N)
                hmr1 = lay.tile([H, N], BF16, tag="hmr1")
                nc.vector.tensor_copy(hmr1[:], t_h[:, hsl])
                # layer 1; produce block 0 before the previous crystal's
                # deferred node update so PE stays ahead of ACT across the
                # boundary (hmr1 only depends on this crystal's h slice)
                aggT1 = lay.tile([H, N], F32, tag="agg1")
                gt10 = gate_produce(0, blocks[0], jp)
                if deferred is not None:
                    finish_crystal(deferred)
                    deferred = None
                gate_consume(gt10, blocks[0], jp, hmr1, aggT1)
                for b in range(1, nblk):
                    gt = gate_produce(0, blocks[b], jp)
                    gate_consume(gt, blocks[b], jp, hmr1, aggT1,
                                 split=(b == nblk - 1))
                # layer 2: produce first two blocks before layer-1 node
                # update so ACT stays busy over the layer-1 reduce tail
                aggT2 = lay.tile([H, N], F32, tag="agg2")
                gt20 = gate_produce(1, blocks[0], jp)
                gt21 = gate_produce(1, blocks[1], jp)
                node_update(c, 0, aggT1, lp)
                hmr2 = lay.tile([H, N], BF16, tag="hmr2")
                nc.vector.tensor_copy(hmr2[:], t_h[:, hsl])
                gate_consume(gt20, blocks[0], jp, hmr2, aggT2)
                gate_consume(gt21, blocks[1], jp, hmr2, aggT2)
                for b in range(2, nblk):
                    gt = gate_produce(1, blocks[b], jp)
                    gate_consume(gt, blocks[b], jp, hmr2, aggT2,
                                 split=(b == nblk - 1))
                deferred = (c, aggT2, lp)

            finish_crystal(deferred)
            nc.gpsimd.dma_start(out=d_sumh[:], in_=sumh[:])

    return nc


def _get_nc(lengths):
    key = tuple(int(x) for x in lengths)
    if key not in _CACHE:
        _CACHE[key] = _build_nc(key)
    return _CACHE[key]


def _shared_inputs(edge_w, edge_b, node_w, node_b):
    centers = np.linspace(0.0, VMAX, BINS).astype(np.float64)
    # cE64: 16 per-fill selector blocks. Fill f uses rf rows 4f..4f+3:
    # row 4f+2g+0 = d^2 of group g, row 4f+2g+1 = d of group g.
    cE = np.zeros((64, NFILL * H), np.float32)
    for f in range(NFILL):
        for g in range(G):
            col0 = f * H + 64 * g
            cE[4 * f + 2 * g + 0, col0:col0 + BINS] = -GAMMA
            cE[4 * f + 2 * g + 1, col0:col0 + BINS] = 2.0 * GAMMA * centers
    cbias = np.zeros((H, 1), np.float32)
    ewR = np.zeros((H, NL * H), np.float32)
    for g in range(G):
        cbias[64 * g:64 * g + BINS, 0] = -GAMMA * centers * centers
        for l in range(NL):
            ewR[64 * g:64 * g + BINS, l * H:(l + 1) * H] = edge_w[l]
    ewR = ewR.astype(ml_dtypes.bfloat16)
    ebT = np.ascontiguousarray(edge_b.T).astype(np.float32)      # [H, NL]
    nwT = np.concatenate([node_w[l] for l in range(NL)], axis=1)
    nwT = np.ascontiguousarray(nwT).astype(np.float32)           # [H, NL*H]
    nbT = np.ascontiguousarray(node_b.T).astype(np.float32)      # [H, NL]
    return dict(cE64=cE, cbias=cbias, ewR=ewR, ebT=ebT, nwT=nwT, nbT=nbT,
                nbTn=-nbT)


def make_in_maps(atom_types, frac_coords, lattice, mask, emb_table,
                 edge_w, edge_b, node_w, node_b):
    shared = _shared_inputs(edge_w, edge_b, node_w, node_b)
    lengths = mask.sum(1).astype(int)
    cart = np.einsum('bnd,bde->bne', frac_coords, lattice).astype(np.float32)
    nsq = (cart * cart).sum(-1)                                   # (B, N)
    d2 = (nsq[:, :, None] + nsq[:, None, :]
          - 2.0 * np.einsum('bid,bjd->bij', cart, cart))
    d2 = np.maximum(d2, 0.0).astype(np.float32) + np.float32(1e-6)
    d = np.sqrt(d2)
    # rf64 [64, sum(8*jp_c)]: crystal c at its packed offset; fill f rows
    # 4f+2g+{0,1} = (d^2, d) of group g, i-rows [8f, 8f+8), j < jp_c,
    # row-major over (i, j).
    offs = [0]
    jps = []
    for c in range(B):
        jp, _, _ = _crystal_geom(lengths[c])
        jps.append(jp)
        offs.append(offs[-1] + IPF * jp)
    rf = np.zeros((64, offs[-1]), np.float32)
    for c in range(B):
        jp = jps[c]
        fw = IPF * jp
        csl = slice(offs[c], offs[c] + fw)
        for f in range(NFILL):
            for g in range(G):
                i0 = g * IPG + f * IPF
                rf[4 * f + 2 * g + 0, csl] = \
                    d2[c, i0:i0 + IPF, :jp].reshape(-1)
                rf[4 * f + 2 * g + 1, csl] = \
                    d[c, i0:i0 + IPF, :jp].reshape(-1)
    types = np.where(mask, atom_types, 0).astype(np.int64)        # (B, N)
    h0 = emb_table[types]                                         # (B, N, H)
    h0T = np.ascontiguousarray(
        h0.transpose(2, 0, 1).reshape(H, B * N)).astype(np.float32)
    maskR = mask.astype(np.float32).reshape(1, B * N).astype(
        ml_dtypes.bfloat16)
    return [dict(rf64=rf, h0T=h0T, maskR=maskR, **shared)]


def _ensure_ntff_hook():
    """run_bass_kernel_spmd(trace=True) imports antenv.axon_hooks, which
    some agent images lack. If it's missing, register the equivalent hook
    from the boot module so a BASS_TRACE=1 run profiles instead of
    crashing. No-op when the real module exists."""
    import sys as _sys
    try:
        import antenv.axon_hooks  # noqa: F401
        return
    except ImportError:
        pass
    try:
        import types as _types
        import antenv  # noqa: F401
        import trn_agent_boot.trn_boot as _tb
        hook = _tb._ntff_profile_via_ctypes('/opt/axon/libaxon_pjrt.so')
        mod = _types.ModuleType('antenv.axon_hooks')
        mod.get_axon_ntff_profile_hook = lambda: hook
        mod.set_axon_ntff_profile_hook = lambda h: None
        _sys.modules['antenv.axon_hooks'] = mod
    except Exception:
        pass


def kernel(**inputs):
    from concourse.bass_utils import run_bass_kernel_spmd

    _ensure_ntff_hook()

    atom_types = np.asarray(inputs["atom_types"])
    frac_coords = np.asarray(inputs["frac_coords"], np.float32)
    lattice = np.asarray(inputs["lattice"], np.float32)
    mask = np.asarray(inputs["mask"]).astype(bool)
    emb_table = np.asarray(inputs["emb_table"], np.float32)
    edge_w = np.asarray(inputs["edge_w"], np.float32)
    edge_b = np.asarray(inputs["edge_b"], np.float32)
    node_w = np.asarray(inputs["node_w"], np.float32)
    node_b = np.asarray(inputs["node_b"], np.float32)
    mu_w = np.asarray(inputs["mu_w"], np.float32)
    mu_b = np.asarray(inputs["mu_b"], np.float32)
    var_w = np.asarray(inputs["var_w"], np.float32)
    var_b = np.asarray(inputs["var_b"], np.float32)

    lengths = mask.sum(1).astype(int)
    nc = _get_nc(lengths)
    in_maps = make_in_maps(atom_types, frac_coords, lattice, mask, emb_table,
                           edge_w, edge_b, node_w, node_b)
    res = run_bass_kernel_spmd(nc, in_maps, core_ids=[0])
    sum_h = np.ascontiguousarray(res.results[0]["sumh"].T)        # (B, H)
    n_valid = mask.sum(1).astype(np.float32)
    g = sum_h / (n_valid[:, None] + 1e-6)
    mu = (g @ mu_w + mu_b).astype(np.float32)
    log_var = (g @ var_w + var_b).astype(np.float32)
    return mu, log_var
